# revision 83
# baseline (speedup 1.0000x reference)
"""Mamba block (RMSNorm -> in_proj -> causal conv1d -> selective scan -> out_proj)
for Trainium2, SPMD over 8 NeuronCores.

Sharding: batch (2) x d_inner (2048 -> 4 slices of 512).
  core c: batch c//4, channels [512*(c%4), 512*(c%4)+512).
Each core computes its partial out_proj contribution; the host sums the 4
partials per batch and stacks batches.  A small on-device AllReduce (96x1024)
merges the x_proj partial sums across the 4 cores of each batch.

Engine plan (per core):
  PE   : transposes (bf16), in_proj/x_proj/dt_proj matmuls (bf16),
         scan y-accumulate via identity matmuls, out_proj (f32r).
  Act  : rms squares + ln/exp, psum evacs, silus, softplus, 64 dA exps.
  DVE  : rms eps/recip, x-evac scale, conv j0/j1, softplus +1, dtxc,
         dBu and prod multiplies (bf16 2x mode), 8 scans, y2.
  Pool : z-evac scale, conv j2/j3, 56 scans (scan runs at 0.6 eff vs
         TT-mult's 0.42, so Pool scans and DVE multiplies).
norm_weight is folded into W_in host-side; the 1/(rms+eps) per-token scale is
applied at in_proj PSUM evacuation via a broadcast tile (DRAM bounce).
"""

import math
import sys

import numpy as np

sys.path.insert(0, "/opt/trn_rl_repo")

D_MODEL = 1024
D_STATE = 16
D_CONV = 4
D_INNER = 2048
DT_RANK = 64  # ceil(1024/16)
EPS = 1e-5

T = 1024          # tokens per batch
CH = 512          # channels per core
NCT = CH // 128   # channel tiles per core (4)
NKT = D_MODEL // 128  # dmodel tiles (8)
N_DVE_SCAN = 2    # states per j whose scan runs on DVE (rest on Pool)

_CACHE = {}
_PHASE_MARKS = []


def _build_program(profile_mode=False, debug=False):
    from contextlib import ExitStack

    import concourse.bacc as bacc
    import concourse.bass as bass
    import concourse.tile as tile
    from concourse import mybir

    f32 = mybir.dt.float32
    f32r = mybir.dt.float32r
    bf16 = mybir.dt.bfloat16
    AF = mybir.ActivationFunctionType
    OP = mybir.AluOpType

    nc = bacc.Bacc("TRN2", target_bir_lowering=False, debug=False, num_devices=8)
    _PHASE_MARKS.clear()
    def _mark(p):
        _PHASE_MARKS.append((p, nc.next_id()))

    hid_t = nc.dram_tensor("hid_bf", [T, D_MODEL], bf16, kind="ExternalInput")
    w_in_t = nc.dram_tensor("w_in_bf", [D_MODEL, 2 * CH], bf16, kind="ExternalInput")
    w_xp_t = nc.dram_tensor("w_xp_bf", [CH, 96], bf16, kind="ExternalInput")
    w_dt_t = nc.dram_tensor("w_dt_bf", [DT_RANK, CH], bf16, kind="ExternalInput")
    w_out_t = nc.dram_tensor("w_out_T", [CH, D_MODEL], f32r, kind="ExternalInput")
    cpack_t = nc.dram_tensor("cpack", [CH, 23], f32, kind="ExternalInput")
    ident_bf = nc.dram_tensor("ident_bf", [128, 128], bf16, kind="ExternalInput")
    # conv taps as diagonal matrices: row p holds diag(conv_w[j*128+p, kk])
    # packed so tile[:, j, kk, :] is the lhsT for tap kk of channel tile j
    diag_cw = nc.dram_tensor("diag_cw", [128, NCT * D_CONV * 128], bf16,
                             kind="ExternalInput")

    part_out = nc.dram_tensor("part_out", [D_MODEL, T], bf16, kind="ExternalOutput")
    if debug:
        dbg = {
            "d_invbc": nc.dram_tensor("d_invbc", [128, T], f32, kind="ExternalOutput"),
            "d_hT": nc.dram_tensor("d_hT", [128, NKT, T], bf16, kind="ExternalOutput"),
            "d_x0": nc.dram_tensor("d_x0", [128, T + 3], f32, kind="ExternalOutput"),
            "d_xc0": nc.dram_tensor("d_xc0", [128, T], f32, kind="ExternalOutput"),
            "d_sz0": nc.dram_tensor("d_sz0", [128, T], f32, kind="ExternalOutput"),
            "d_xdbl": nc.dram_tensor("d_xdbl", [96, T], bf16, kind="ExternalOutput"),
            "d_dt": nc.dram_tensor("d_dt", [128, NCT, T], f32, kind="ExternalOutput"),
            "d_y20": nc.dram_tensor("d_y20", [128, T], f32r, kind="ExternalOutput"),
            "d_ss": nc.dram_tensor("d_ss", [128, NKT], f32, kind="ExternalOutput"),
        }

    inv_dram = nc.dram_tensor("inv_dram", [T], f32)
    cc_in = nc.dram_tensor("cc_in", [2, 96, 512], bf16)
    cc_out = nc.dram_tensor("cc_out", [2, 96, 512], bf16)

    RG = [[0, 1, 2, 3], [4, 5, 6, 7]]

    with tile.TileContext(nc) as tc, ExitStack() as ctx:
        consts = ctx.enter_context(tc.tile_pool(name="consts", bufs=1))
        persist = ctx.enter_context(tc.tile_pool(name="persist", bufs=1))

        # ---- constant + input loads (HWDGE is a single-slot resource:
        # emission order = dispatch order; identbf first for transposes) ----
        identbf_sb = consts.tile([128, 128], bf16, tag="identbf")
        nc.sync.dma_start(out=identbf_sb[:], in_=ident_bf.ap())

        # persistent activations (only those read by the scan/out_proj)
        xcbf_sb = [persist.tile([128, T], bf16, tag=f"xcb{j}", name=f"xcb{j}") for j in range(NCT)]
        sz_sb = persist.tile([128, NCT, T], f32, tag="sz")
        dt_sb = persist.tile([128, NCT, T], f32, tag="dt")
        dtxc_sb = [persist.tile([128, T], bf16, tag=f"dtxc{j}", name=f"dtxc{j}") for j in range(NCT)]
        y2_sb = [persist.tile([128, T], f32r, tag=f"y2{j}", name=f"y2{j}") for j in range(NCT)]

        _mark("consts")
        with ExitStack() as front:
            hidp = front.enter_context(tc.tile_pool(name="hidp", bufs=1))
            winp = front.enter_context(tc.tile_pool(name="winp", bufs=1))
            sst = front.enter_context(tc.tile_pool(name="sst", bufs=1))
            scr = front.enter_context(tc.tile_pool(name="scr", bufs=1))
            fper = front.enter_context(tc.tile_pool(name="fper", bufs=1))

            hTbf = fper.tile([128, NKT, T], bf16, tag="hT", name="hT")
            x_sb = [fper.tile([128, T + D_CONV - 1], bf16, tag=f"x{j}", name=f"x{j}")
                    for j in range(NCT)]
            diag_sb = fper.tile([128, NCT, D_CONV, 128], bf16, tag="diagcw")
            xdbl_sb = fper.tile([96, T], bf16, tag="xdbl")
            inv_bc = fper.tile([128, T], f32, tag="invbc")
            for j in range(NCT):
                nc.vector.memset(x_sb[j][:, 0:D_CONV - 1], 0.0)

            hid_sb = hidp.tile([128, NKT, T], bf16, tag="hid")
            win_sb = winp.tile([128, NKT, 2 * CH], bf16, tag="win")
            # hid halves [128,4,1024]; w_in split x-cols/z-cols so the h0
            # x-chains can start before the z columns arrive
            def hid_dma(hh):
                nc.sync.dma_start(
                    out=hid_sb[:, hh * 4:(hh + 1) * 4, :],
                    in_=bass.AP(tensor=hid_t, offset=hh * 4 * 128 * D_MODEL,
                                ap=[[D_MODEL, 128], [128 * D_MODEL, 4], [1, D_MODEL]]),
                )

            def win_dma(hh):
                nc.sync.dma_start(
                    out=win_sb[:, :, hh * 512:(hh + 1) * 512],
                    in_=bass.AP(tensor=w_in_t, offset=hh * 512,
                                ap=[[2 * CH, 128], [128 * 2 * CH, NKT], [1, 512]]))

            hid_dma(0)
            win_dma(0)
            cpk_sb = consts.tile([128, NCT, 23], f32, tag="cpk")
            nc.sync.dma_start(out=cpk_sb[:],
                              in_=cpack_t.ap().rearrange("(j p) n -> p j n", p=128))
            wxp_sb = consts.tile([128, NCT, 96], bf16, tag="wxp")
            nc.sync.dma_start(out=wxp_sb[:],
                              in_=w_xp_t.ap().rearrange("(j p) n -> p j n", p=128))
            wdt_sb = consts.tile([DT_RANK, CH], bf16, tag="wdt")
            nc.sync.dma_start(out=wdt_sb[:], in_=w_dt_t.ap())
            hid_dma(1)
            nc.sync.dma_start(out=diag_sb[:], in_=diag_cw.ap())
            win_dma(1)
            a_sb = cpk_sb[:, :, 0:16]
            cw_sb = cpk_sb[:, :, 16:20]
            cb_sb = cpk_sb[:, :, 20:21]
            dtb_sb = cpk_sb[:, :, 21:22]
            d_sb = cpk_sb[:, :, 22:23]

            # ---- rms stats + transposes (per hid t-tile i) ----
            scr_t = scr.tile([128, T], bf16, tag="scr")
            ss_all = sst.tile([128, NKT], f32, tag="ss")
            with ExitStack() as ptp:
                psM = ptp.enter_context(tc.tile_pool(name="psM", bufs=3, space="PSUM"))
                psC = ptp.enter_context(tc.tile_pool(name="psC", bufs=1, space="PSUM"))
                zt_p = ptp.enter_context(tc.tile_pool(name="ztp", bufs=2))
                psX = ptp.enter_context(tc.tile_pool(name="psX", bufs=1, space="PSUM"))
                bpool = ptp.enter_context(tc.tile_pool(name="bpool", bufs=2))
                ps96 = psX.tile([96, T], f32, tag="ps96")
                tps = ptp.enter_context(ExitStack())
                psT = tps.enter_context(tc.tile_pool(name="psT", bufs=2, space="PSUM"))

                def do_square(i):
                    nc.scalar.activation(scr_t[:], hid_sb[:, i, :], AF.Square,
                                         accum_out=ss_all[:, i:i + 1])

                def do_transpose(i):
                    for gh in range(2):
                        pst = psT.tile([128, 512], bf16, tag="psT")
                        for q in range(4):
                            g = gh * 4 + q
                            nc.tensor.transpose(
                                pst[:, q * 128:(q + 1) * 128],
                                hid_sb[:, i, g * 128:(g + 1) * 128], identbf_sb[:])
                        # DVE copy (bf16 2x): Act is busy with the squares
                        nc.vector.tensor_copy(
                            out=hTbf[:, gh * 4:(gh + 1) * 4, i * 128:(i + 1) * 128],
                            in_=pst[:])

                for i in range(4):
                    do_square(i)
                    do_transpose(i)

                _mark("rmsT0")
                # rms tail per t-half: inv for t in half h needs only hid
                # tiles h*4..h*4+3 (keeps the half-pipelined in_proj legal).
                # 1/(rms+eps) ~= rsqrt(ms) to ~1e-5 rel; single Act op whose
                # table set also holds Square (no reload between them).
                def rms_tail(h):
                    hs = slice(h * 4, (h + 1) * 4)
                    den = sst.tile([128, 4], f32, tag=f"den{h}")
                    nc.scalar.activation(den[:], ss_all[:, hs], AF.Sqrt,
                                         scale=1.0 / D_MODEL)
                    den2 = sst.tile([128, 4], f32, tag=f"den2{h}")
                    nc.vector.tensor_scalar_add(den2[:], den[:], EPS)
                    inv = sst.tile([128, 4], f32, tag=f"inv{h}")
                    nc.vector.reciprocal(inv[:], den2[:])
                    nc.sync.dma_start(
                        out=bass.AP(tensor=inv_dram, offset=h * 512,
                                    ap=[[1, 128], [128, 4]]),
                        in_=inv[:])
                    nc.sync.dma_start(
                        out=inv_bc[:, h * 512:(h + 1) * 512],
                        in_=bass.AP(tensor=inv_dram, offset=h * 512,
                                    ap=[[0, 128], [1, 512]]))

                def conv_j(j, h):
                    # causal depthwise conv as 4 PE diag-matmuls into PSUM
                    cs = slice(h * 512, (h + 1) * 512)
                    pc = psC.tile([128, 512], f32, tag="psC")
                    for kk in range(D_CONV):
                        nc.tensor.matmul(
                            pc[:],
                            lhsT=diag_sb[:, j, kk, :],
                            rhs=x_sb[j][:, kk + h * 512:kk + h * 512 + 512],
                            start=(kk == 0), stop=(kk == D_CONV - 1))
                    nc.scalar.activation(xcbf_sb[j][:, cs], pc[:], AF.Silu,
                                         bias=cb_sb[:, j, :])

                def mchain(m, h):
                    cs = slice(h * 512, (h + 1) * 512)
                    ps = psM.tile([128, 512], f32, tag="psM")
                    for k in range(NKT):
                        nc.tensor.matmul(
                            ps[:],
                            lhsT=win_sb[:, k, m * 128:(m + 1) * 128],
                            rhs=hTbf[:, k, cs],
                            start=(k == 0), stop=(k == NKT - 1))
                    return ps

                def x_half(h):
                    # x m-chains with lag-1 convs (j never stalls on its evac)
                    cs = slice(h * 512, (h + 1) * 512)
                    for m in range(4):
                        ps = mchain(m, h)
                        nc.vector.tensor_tensor(
                            out=x_sb[m][:, D_CONV - 1 + h * 512:D_CONV - 1 + (h + 1) * 512],
                            in0=ps[:], in1=inv_bc[:, cs], op=OP.mult)
                        if m >= 1:
                            conv_j(m - 1, h)

                def xproj_cc(h):
                    cs = slice(h * 512, (h + 1) * 512)
                    conv_j(3, h)
                    for k in range(NCT):
                        nc.tensor.matmul(
                            ps96[:, cs],
                            lhsT=wxp_sb[:, k, :],
                            rhs=xcbf_sb[k][:, cs],
                            start=(k == 0), stop=(k == NCT - 1))
                    xdbl_part = bpool.tile([96, 512], bf16, tag=f"xdblp{h}")
                    nc.scalar.activation(xdbl_part[:], ps96[:, cs], AF.Copy)
                    # h1 chain on the Pool queue (idle until the scan) so the
                    # two allreduce chains don't serialize on SP
                    deng = nc.sync if h == 0 else nc.gpsimd
                    deng.dma_start(out=cc_in[h], in_=xdbl_part[:])
                    if profile_mode:
                        deng.dma_start(out=cc_out[h], in_=cc_in[h])
                    else:
                        nc.gpsimd.collective_compute(
                            "AllReduce", mybir.AluOpType.add, replica_groups=RG,
                            ins=[cc_in[h]], outs=[cc_out[h]])
                    deng.dma_start(out=xdbl_sb[:, cs], in_=cc_out[h])

                def z_half(h):
                    # write scaled z into sz; the Silu runs later in one
                    # batch (between softplus Lns and dA Exps) so its table
                    # loads never interleave with the Exp/Ln block
                    cs = slice(h * 512, (h + 1) * 512)
                    for m in range(4, 8):
                        ps = mchain(m, h)
                        j = m - 4
                        nc.vector.tensor_tensor(out=sz_sb[:, j, cs], in0=ps[:],
                                                in1=inv_bc[:, cs], op=OP.mult)

                rms_tail(0)
                for i in range(4, 8):
                    do_square(i)
                rms_tail(1)
                _mark("rmsT1")
                x_half(0)
                for i in range(4, 8):
                    do_transpose(i)
                tps.close()  # free psT banks for psD
                xproj_cc(0)
                _mark("inproj0")
                x_half(1)
                xproj_cc(1)
                _mark("inproj1")
                if debug:
                    nc.sync.dma_start(out=dbg["d_ss"].ap(), in_=ss_all[:])

                _mark("xproj")
                psD = ptp.enter_context(tc.tile_pool(name="psD", bufs=2, space="PSUM"))
                spool = ptp.enter_context(tc.tile_pool(name="spool", bufs=2))
                p1p = ptp.enter_context(tc.tile_pool(name="p1p", bufs=1))

                # dt_proj + softplus: softplus(u+b) = ln(1+exp(u+b)).
                p1v = [p1p.tile([128, T], f32, tag=f"p1v{j}", name=f"p1v{j}")
                       for j in range(NCT)]
                for h in range(2):
                    cs = slice(h * 512, (h + 1) * 512)
                    for j in range(NCT):
                        psd = psD.tile([128, 512], f32, tag="psD")
                        nc.tensor.matmul(
                            psd[:],
                            lhsT=wdt_sb[:, j * 128:(j + 1) * 128],
                            rhs=xdbl_sb[0:DT_RANK, cs],
                            start=True, stop=True)
                        e1 = spool.tile([128, 512], f32, tag="sp_e")
                        nc.scalar.activation(e1[:], psd[:], AF.Exp,
                                             bias=dtb_sb[:, j, :])
                        nc.vector.tensor_scalar_add(p1v[j][:, cs], e1[:], 1.0)
                for j in range(NCT):
                    for h in range(2):
                        cs = slice(h * 512, (h + 1) * 512)
                        nc.scalar.activation(dt_sb[:, j, cs], p1v[j][:, cs], AF.Ln)
                    nc.vector.tensor_tensor(
                        out=dtxc_sb[j][:], in0=dt_sb[:, j, :],
                        in1=xcbf_sb[j][:], op=OP.mult)
                # z m-chains after the dt block: lower PE priority, so the
                # psd matmuls preempt them when each allreduce half returns
                z_half(0)
                z_half(1)

            _mark("dt")
            if debug:
                nc.sync.dma_start(out=dbg["d_invbc"].ap(), in_=inv_bc[:])
                nc.sync.dma_start(out=dbg["d_hT"].ap(), in_=hTbf[:])
                nc.sync.dma_start(out=dbg["d_x0"].ap(), in_=x_sb[0][:])
                nc.sync.dma_start(out=dbg["d_xdbl"].ap(), in_=xdbl_sb[:])
        # ===================== scan block (n-pair outer, j-inner) =========
        with ExitStack() as back:
            woutp = back.enter_context(tc.tile_pool(name="woutp", bufs=1))
            wout_sb = woutp.tile([128, NCT, D_MODEL], f32r, tag="wout")

            p6 = back.enter_context(ExitStack())
            bcp = p6.enter_context(tc.tile_pool(name="bcp", bufs=3))
            dAp = p6.enter_context(tc.tile_pool(name="dAp", bufs=3))
            dBup = p6.enter_context(tc.tile_pool(name="dBup", bufs=3))
            hp = p6.enter_context(tc.tile_pool(name="hp", bufs=4))
            prp = p6.enter_context(tc.tile_pool(name="prp", bufs=3))
            gat = p6.enter_context(tc.tile_pool(name="gat", bufs=2))
            psY = p6.enter_context(tc.tile_pool(name="psY", bufs=1, space="PSUM"))

            psy = [psY.tile([128, T], f32, tag=f"psy{j}", name=f"psy{j}")
                   for j in range(NCT)]

            # States processed in pairs: [128, 2T] tiles with a zeroed seam
            # column resetting the scan at the segment boundary.
            NP = D_STATE // 2
            NPRE = 2
            bc_tiles = {}

            def load_bc(p):
                t = bcp.tile([128, 2, 2 * T], bf16, tag="bc")
                # free layout per B/C: [n-pair(2) x t-half(2) x 512];
                # broadcast rows of cc_out[2,96,512] to 128 partitions
                for bc_i in range(2):
                    base = t[:, bc_i, :]
                    for h in range(2):
                        nc.sync.dma_start(
                            out=bass.AP(tensor=base.tensor,
                                        offset=base.offset + h * 512,
                                        ap=[base.ap[0], [T, 2], [1, 512]]),
                            in_=bass.AP(
                                tensor=cc_out,
                                offset=96 * 512 * h + (64 + 16 * bc_i + 2 * p) * 512,
                                ap=[[0, 128], [512, 2], [1, 512]]))
                bc_tiles[p] = t

            def rep2(ap):
                return bass.AP(tensor=ap.tensor, offset=ap.offset,
                               ap=[ap.ap[0], [0, 2], ap.ap[1]])

            for p in range(NPRE):
                load_bc(p)

            # Scans are DVE-only (codegen rejects TensorScalarPtr on Pool).
            # DVE: dBu + scan (+ a few prods); Pool: most prods (plain TT).
            n_pend = [0]

            def emit_prod(j, p, hsc):
                prod = prp.tile([128, 2 * T], bf16, tag="prod")
                idx = n_pend[0]; n_pend[0] += 1
                peng = nc.vector if idx % 10 == 9 else nc.gpsimd
                peng.tensor_tensor(out=prod[:], in0=hsc[:],
                                   in1=bc_tiles[p][:, 1, :], op=OP.mult)
                for s in range(2):
                    n = 2 * p + s
                    for h in range(2):
                        nc.tensor.matmul(
                            psy[j][:, h * 512:(h + 1) * 512],
                            lhsT=identbf_sb[:],
                            rhs=prod[:, s * T + h * 512:s * T + (h + 1) * 512],
                            start=(n == 0), stop=(n == D_STATE - 1))

            for p in range(NP):
                if p + NPRE < NP:
                    load_bc(p + NPRE)
                if p == 3:
                    # deferred z-gates: one Silu batch, in place, emitted
                    # once all softplus Exp/Ln work has long retired so the
                    # scheduler can't interleave it into that table set
                    nc.scalar.activation(sz_sb[:], sz_sb[:], AF.Silu)
                if p == 2:
                    # out_proj weights after the early B/C prefetches
                    nc.sync.dma_start(
                        out=wout_sb[:],
                        in_=w_out_t.ap().rearrange("(k p) n -> p k n", p=128))
                for j in range(NCT):
                    dA = dAp.tile([128, 2 * T], bf16, tag="dA")
                    for s in range(2):
                        nc.scalar.activation(dA[:, s * T:(s + 1) * T],
                                             dt_sb[:, j, :], AF.Exp,
                                             scale=a_sb[:, j, 2 * p + s:2 * p + s + 1])
                    nc.vector.memset(dA[:, T:T + 1], 0.0)  # segment seam
                    dBu = dBup.tile([128, 2 * T], bf16, tag="dBu")
                    nc.vector.tensor_tensor(out=dBu[:], in0=rep2(dtxc_sb[j][:]),
                                            in1=bc_tiles[p][:, 0, :], op=OP.mult)
                    hsc = hp.tile([128, 2 * T], bf16, tag="h")
                    nc.vector.tensor_tensor_scan(
                        hsc[:], dA[:], dBu[:], 0.0, OP.mult, OP.add)
                    emit_prod(j, p, hsc)

            first = True
            for h in range(2):
                cs = slice(h * 512, (h + 1) * 512)
                for j in range(NCT):
                    t1 = gat.tile([128, 512], f32, tag="t1")
                    nc.vector.scalar_tensor_tensor(
                        out=t1[:], in0=xcbf_sb[j][:, cs], scalar=d_sb[:, j, :],
                        in1=psy[j][:, cs], op0=OP.mult, op1=OP.add)
                    # y2 on Pool (free at scan end) halves the PE-idle gap
                    nc.gpsimd.tensor_tensor(out=y2_sb[j][:, cs], in0=t1[:],
                                            in1=sz_sb[:, j, cs], op=OP.mult)
                    if first:
                        # keep the PE p-state hot through the y2 window by
                        # recycling the consumed psy[0] bank as scratch
                        first = False
                        for w in range(16):
                            nc.tensor.matmul(
                                psy[0][:, 0:512], lhsT=identbf_sb[:],
                                rhs=dtxc_sb[0][:, 0:512],
                                start=True, stop=True)

            _mark("scan")
            if debug:
                nc.sync.dma_start(out=dbg["d_sz0"].ap(), in_=sz_sb[:, 0, :])
                nc.sync.dma_start(out=dbg["d_dt"].ap(), in_=dt_sb[:])
                nc.sync.dma_start(out=dbg["d_y20"].ap(), in_=y2_sb[0][:])
            p6.close()  # frees scan pools + psY banks before out_proj
            # ================== out_proj partial ======================
            with ExitStack() as p7:
                psO = p7.enter_context(tc.tile_pool(name="psO", bufs=4, space="PSUM"))
                oev = p7.enter_context(tc.tile_pool(name="oev", bufs=2))
                for mp in range(4):  # m-pairs
                    ot = oev.tile([128, 2, T], bf16, tag="oev")
                    for mi in range(2):
                        m = mp * 2 + mi
                        for h in range(2):
                            pso = psO.tile([128, 512], f32, tag="psO")
                            for k in range(NCT):
                                nc.tensor.matmul(
                                    pso[:],
                                    lhsT=wout_sb[:, k, m * 128:(m + 1) * 128],
                                    rhs=y2_sb[k][:, h * 512:(h + 1) * 512],
                                    start=(k == 0), stop=(k == NCT - 1))
                            nc.scalar.activation(
                                ot[:, mi, h * 512:(h + 1) * 512], pso[:], AF.Copy)
                    nc.sync.dma_start(
                        out=bass.AP(tensor=part_out, offset=mp * 2 * 128 * T,
                                    ap=[[T, 128], [128 * T, 2], [1, T]]),
                        in_=ot[:])

    _mark("out_proj")
    nc.compile()
    return nc


def _get_program():
    if "nc" not in _CACHE:
        _CACHE["nc"] = _build_program()
    return _CACHE["nc"]


def kernel(hidden_states, norm_weight, in_proj_w, conv_w, conv_b, x_proj_w,
           dt_proj_w, dt_proj_b, A_log, D, out_proj_w):
    from concourse.bass_utils import run_bass_kernel_spmd
    import ml_dtypes

    bf = ml_dtypes.bfloat16

    hidden_states = np.asarray(hidden_states, dtype=np.float32)
    norm_weight = np.asarray(norm_weight, dtype=np.float32)
    in_proj_w = np.asarray(in_proj_w, dtype=np.float32)
    conv_w = np.asarray(conv_w, dtype=np.float32)
    conv_b = np.asarray(conv_b, dtype=np.float32)
    x_proj_w = np.asarray(x_proj_w, dtype=np.float32)
    dt_proj_w = np.asarray(dt_proj_w, dtype=np.float32)
    dt_proj_b = np.asarray(dt_proj_b, dtype=np.float32)
    A_log = np.asarray(A_log, dtype=np.float32)
    D = np.asarray(D, dtype=np.float32)
    out_proj_w = np.asarray(out_proj_w, dtype=np.float32)

    nc = _get_program()

    a_neg_full = -np.exp(A_log)  # [2048, 16]
    ident_bf = np.eye(128, dtype=bf)
    eye128 = np.eye(128, dtype=np.float32)

    in_maps = []
    for c in range(8):
        b, j = c // 4, c % 4
        sl = slice(CH * j, CH * (j + 1))
        w_in_cat = np.concatenate(
            [in_proj_w[sl], in_proj_w[D_INNER + CH * j:D_INNER + CH * (j + 1)]],
            axis=0)  # [2CH, D_MODEL]
        w_in_fold = w_in_cat * norm_weight[None, :]
        cpack = np.concatenate(
            [a_neg_full[sl], conv_w[sl], conv_b[sl, None], dt_proj_b[sl, None],
             D[sl, None]], axis=1).astype(np.float32)
        # diag_cw[p, j*512 + kk*128 + c] = conv_w[j*128+p, kk] * (c == p)
        dk = conv_w[sl].reshape(NCT, 128, D_CONV)            # [j, p, kk]
        diag = np.einsum('jpk,pc->pjkc', dk, eye128)         # [p, j, kk, c]
        in_maps.append({
            "hid_bf": hidden_states[b].astype(bf),
            "w_in_bf": np.ascontiguousarray(w_in_fold.T).astype(bf),
            "w_xp_bf": np.ascontiguousarray(x_proj_w[:, sl].T).astype(bf),
            "w_dt_bf": np.ascontiguousarray(dt_proj_w[sl, :].T).astype(bf),
            "w_out_T": np.ascontiguousarray(out_proj_w[:, sl].T),
            "cpack": np.ascontiguousarray(cpack),
            "ident_bf": ident_bf,
            "diag_cw": np.ascontiguousarray(diag.reshape(128, -1)).astype(bf),
        })

    import os
    kw = {}
    if os.environ.get("MAMBA_TRACE"):
        kw = dict(trace=True, tmpdir=os.environ.get("MAMBA_TRACE_DIR") or None)
    res = run_bass_kernel_spmd(nc, in_maps, list(range(8)), **kw)
    _CACHE["last_results"] = res

    out = np.zeros((2, T, D_MODEL), np.float32)
    for c in range(8):
        b = c // 4
        out[b] += res.results[c]["part_out"].T.astype(np.float32)
    return out, hidden_states


# revision 84
# speedup vs baseline: 1.0117x; 1.0117x over previous
"""Mamba block (RMSNorm -> in_proj -> causal conv1d -> selective scan -> out_proj)
for Trainium2, SPMD over 8 NeuronCores.

Sharding: batch (2) x d_inner (2048 -> 4 slices of 512).
  core c: batch c//4, channels [512*(c%4), 512*(c%4)+512).
Each core computes its partial out_proj contribution; the host sums the 4
partials per batch and stacks batches.  A small on-device AllReduce (96x1024)
merges the x_proj partial sums across the 4 cores of each batch.

Engine plan (per core):
  PE   : transposes (bf16), in_proj/x_proj/dt_proj matmuls (bf16),
         scan y-accumulate via identity matmuls, out_proj (f32r).
  Act  : rms squares + ln/exp, psum evacs, silus, softplus, 64 dA exps.
  DVE  : rms eps/recip, x-evac scale, conv j0/j1, softplus +1, dtxc,
         dBu and prod multiplies (bf16 2x mode), 8 scans, y2.
  Pool : z-evac scale, conv j2/j3, 56 scans (scan runs at 0.6 eff vs
         TT-mult's 0.42, so Pool scans and DVE multiplies).
norm_weight is folded into W_in host-side; the 1/(rms+eps) per-token scale is
applied at in_proj PSUM evacuation via a broadcast tile (DRAM bounce).
"""

import math
import sys

import numpy as np

sys.path.insert(0, "/opt/trn_rl_repo")

D_MODEL = 1024
D_STATE = 16
D_CONV = 4
D_INNER = 2048
DT_RANK = 64  # ceil(1024/16)
EPS = 1e-5

T = 1024          # tokens per batch
CH = 512          # channels per core
NCT = CH // 128   # channel tiles per core (4)
NKT = D_MODEL // 128  # dmodel tiles (8)
N_DVE_SCAN = 2    # states per j whose scan runs on DVE (rest on Pool)

_CACHE = {}
_PHASE_MARKS = []


def _build_program(profile_mode=False, debug=False):
    from contextlib import ExitStack

    import concourse.bacc as bacc
    import concourse.bass as bass
    import concourse.tile as tile
    from concourse import mybir

    f32 = mybir.dt.float32
    f32r = mybir.dt.float32r
    bf16 = mybir.dt.bfloat16
    AF = mybir.ActivationFunctionType
    OP = mybir.AluOpType

    nc = bacc.Bacc("TRN2", target_bir_lowering=False, debug=False, num_devices=8)
    _PHASE_MARKS.clear()
    def _mark(p):
        _PHASE_MARKS.append((p, nc.next_id()))

    hid_t = nc.dram_tensor("hid_bf", [T, D_MODEL], bf16, kind="ExternalInput")
    w_in_t = nc.dram_tensor("w_in_bf", [D_MODEL, 2 * CH], bf16, kind="ExternalInput")
    w_xp_t = nc.dram_tensor("w_xp_bf", [CH, 96], bf16, kind="ExternalInput")
    w_dt_t = nc.dram_tensor("w_dt_bf", [DT_RANK, CH], bf16, kind="ExternalInput")
    w_out_t = nc.dram_tensor("w_out_T", [CH, D_MODEL], f32r, kind="ExternalInput")
    cpack_t = nc.dram_tensor("cpack", [CH, 23], f32, kind="ExternalInput")
    ident_bf = nc.dram_tensor("ident_bf", [128, 128], bf16, kind="ExternalInput")
    # conv taps as diagonal matrices: row p holds diag(conv_w[j*128+p, kk])
    # packed so tile[:, j, kk, :] is the lhsT for tap kk of channel tile j
    diag_cw = nc.dram_tensor("diag_cw", [128, NCT * D_CONV * 128], bf16,
                             kind="ExternalInput")

    part_out = nc.dram_tensor("part_out", [D_MODEL, T], bf16, kind="ExternalOutput")
    if debug:
        dbg = {
            "d_invbc": nc.dram_tensor("d_invbc", [128, T], f32, kind="ExternalOutput"),
            "d_hT": nc.dram_tensor("d_hT", [128, NKT, T], bf16, kind="ExternalOutput"),
            "d_x0": nc.dram_tensor("d_x0", [128, T + 3], f32, kind="ExternalOutput"),
            "d_xc0": nc.dram_tensor("d_xc0", [128, T], f32, kind="ExternalOutput"),
            "d_sz0": nc.dram_tensor("d_sz0", [128, T], f32, kind="ExternalOutput"),
            "d_xdbl": nc.dram_tensor("d_xdbl", [96, T], bf16, kind="ExternalOutput"),
            "d_dt": nc.dram_tensor("d_dt", [128, NCT, T], f32, kind="ExternalOutput"),
            "d_y20": nc.dram_tensor("d_y20", [128, T], f32r, kind="ExternalOutput"),
            "d_ss": nc.dram_tensor("d_ss", [128, NKT], f32, kind="ExternalOutput"),
        }

    inv_dram = nc.dram_tensor("inv_dram", [T], f32)
    cc_in = nc.dram_tensor("cc_in", [2, 96, 512], bf16)
    cc_out = nc.dram_tensor("cc_out", [2, 96, 512], bf16)

    RG = [[0, 1, 2, 3], [4, 5, 6, 7]]

    with tile.TileContext(nc) as tc, ExitStack() as ctx:
        consts = ctx.enter_context(tc.tile_pool(name="consts", bufs=1))
        persist = ctx.enter_context(tc.tile_pool(name="persist", bufs=1))

        # ---- constant + input loads (HWDGE is a single-slot resource:
        # emission order = dispatch order; identbf first for transposes) ----
        identbf_sb = consts.tile([128, 128], bf16, tag="identbf")
        nc.sync.dma_start(out=identbf_sb[:], in_=ident_bf.ap())

        # persistent activations (only those read by the scan/out_proj)
        xcbf_sb = [persist.tile([128, T], bf16, tag=f"xcb{j}", name=f"xcb{j}") for j in range(NCT)]
        sz_sb = persist.tile([128, NCT, T], f32, tag="sz")
        dt_sb = persist.tile([128, NCT, T], f32, tag="dt")
        dtxc_sb = [persist.tile([128, T], bf16, tag=f"dtxc{j}", name=f"dtxc{j}") for j in range(NCT)]
        y2_sb = [persist.tile([128, T], f32r, tag=f"y2{j}", name=f"y2{j}") for j in range(NCT)]

        _mark("consts")
        with ExitStack() as front:
            hidp = front.enter_context(tc.tile_pool(name="hidp", bufs=1))
            winp = front.enter_context(tc.tile_pool(name="winp", bufs=1))
            sst = front.enter_context(tc.tile_pool(name="sst", bufs=1))
            scr = front.enter_context(tc.tile_pool(name="scr", bufs=1))
            fper = front.enter_context(tc.tile_pool(name="fper", bufs=1))

            hTbf = fper.tile([128, NKT, T], bf16, tag="hT", name="hT")
            x_sb = [fper.tile([128, T + D_CONV - 1], bf16, tag=f"x{j}", name=f"x{j}")
                    for j in range(NCT)]
            diag_sb = fper.tile([128, NCT, D_CONV, 128], bf16, tag="diagcw")
            xdbl_sb = fper.tile([96, T], bf16, tag="xdbl")
            inv_bc = fper.tile([128, T], f32, tag="invbc")
            for j in range(NCT):
                nc.vector.memset(x_sb[j][:, 0:D_CONV - 1], 0.0)

            hid_sb = hidp.tile([128, NKT, T], bf16, tag="hid")
            win_sb = winp.tile([128, NKT, 2 * CH], bf16, tag="win")
            # hid halves [128,4,1024]; w_in split x-cols/z-cols so the h0
            # x-chains can start before the z columns arrive
            def hid_dma(hh):
                nc.sync.dma_start(
                    out=hid_sb[:, hh * 4:(hh + 1) * 4, :],
                    in_=bass.AP(tensor=hid_t, offset=hh * 4 * 128 * D_MODEL,
                                ap=[[D_MODEL, 128], [128 * D_MODEL, 4], [1, D_MODEL]]),
                )

            def win_dma(hh):
                nc.sync.dma_start(
                    out=win_sb[:, :, hh * 512:(hh + 1) * 512],
                    in_=bass.AP(tensor=w_in_t, offset=hh * 512,
                                ap=[[2 * CH, 128], [128 * 2 * CH, NKT], [1, 512]]))

            hid_dma(0)
            win_dma(0)
            cpk_sb = consts.tile([128, NCT, 23], f32, tag="cpk")
            nc.sync.dma_start(out=cpk_sb[:],
                              in_=cpack_t.ap().rearrange("(j p) n -> p j n", p=128))
            wxp_sb = consts.tile([128, NCT, 96], bf16, tag="wxp")
            nc.sync.dma_start(out=wxp_sb[:],
                              in_=w_xp_t.ap().rearrange("(j p) n -> p j n", p=128))
            wdt_sb = consts.tile([DT_RANK, CH], bf16, tag="wdt")
            nc.sync.dma_start(out=wdt_sb[:], in_=w_dt_t.ap())
            hid_dma(1)
            nc.sync.dma_start(out=diag_sb[:], in_=diag_cw.ap())
            win_dma(1)
            a_sb = cpk_sb[:, :, 0:16]
            cw_sb = cpk_sb[:, :, 16:20]
            cb_sb = cpk_sb[:, :, 20:21]
            dtb_sb = cpk_sb[:, :, 21:22]
            d_sb = cpk_sb[:, :, 22:23]

            # ---- rms stats + transposes (per hid t-tile i) ----
            scr_t = scr.tile([128, T], bf16, tag="scr")
            ss_all = sst.tile([128, NKT], f32, tag="ss")
            with ExitStack() as ptp:
                psM = ptp.enter_context(tc.tile_pool(name="psM", bufs=3, space="PSUM"))
                psC = ptp.enter_context(tc.tile_pool(name="psC", bufs=1, space="PSUM"))
                zt_p = ptp.enter_context(tc.tile_pool(name="ztp", bufs=2))
                psX = ptp.enter_context(tc.tile_pool(name="psX", bufs=1, space="PSUM"))
                bpool = ptp.enter_context(tc.tile_pool(name="bpool", bufs=2))
                ps96 = psX.tile([96, T], f32, tag="ps96")
                tps = ptp.enter_context(ExitStack())
                psT = tps.enter_context(tc.tile_pool(name="psT", bufs=2, space="PSUM"))

                def do_square(i):
                    nc.scalar.activation(scr_t[:], hid_sb[:, i, :], AF.Square,
                                         accum_out=ss_all[:, i:i + 1])

                def do_transpose(i):
                    for gh in range(2):
                        pst = psT.tile([128, 512], bf16, tag="psT")
                        for q in range(4):
                            g = gh * 4 + q
                            nc.tensor.transpose(
                                pst[:, q * 128:(q + 1) * 128],
                                hid_sb[:, i, g * 128:(g + 1) * 128], identbf_sb[:])
                        # DVE copy (bf16 2x): Act is busy with the squares
                        nc.vector.tensor_copy(
                            out=hTbf[:, gh * 4:(gh + 1) * 4, i * 128:(i + 1) * 128],
                            in_=pst[:])

                for i in range(4):
                    do_square(i)
                    do_transpose(i)

                _mark("rmsT0")
                # rms tail per t-half: inv for t in half h needs only hid
                # tiles h*4..h*4+3 (keeps the half-pipelined in_proj legal).
                # 1/(rms+eps) ~= rsqrt(ms) to ~1e-5 rel; single Act op whose
                # table set also holds Square (no reload between them).
                def rms_tail(h):
                    hs = slice(h * 4, (h + 1) * 4)
                    den = sst.tile([128, 4], f32, tag=f"den{h}")
                    nc.scalar.activation(den[:], ss_all[:, hs], AF.Sqrt,
                                         scale=1.0 / D_MODEL)
                    den2 = sst.tile([128, 4], f32, tag=f"den2{h}")
                    nc.vector.tensor_scalar_add(den2[:], den[:], EPS)
                    inv = sst.tile([128, 4], f32, tag=f"inv{h}")
                    nc.vector.reciprocal(inv[:], den2[:])
                    nc.sync.dma_start(
                        out=bass.AP(tensor=inv_dram, offset=h * 512,
                                    ap=[[1, 128], [128, 4]]),
                        in_=inv[:])
                    nc.sync.dma_start(
                        out=inv_bc[:, h * 512:(h + 1) * 512],
                        in_=bass.AP(tensor=inv_dram, offset=h * 512,
                                    ap=[[0, 128], [1, 512]]))

                def conv_j(j, h):
                    # causal depthwise conv as 4 PE diag-matmuls into PSUM
                    cs = slice(h * 512, (h + 1) * 512)
                    pc = psC.tile([128, 512], f32, tag="psC")
                    for kk in range(D_CONV):
                        nc.tensor.matmul(
                            pc[:],
                            lhsT=diag_sb[:, j, kk, :],
                            rhs=x_sb[j][:, kk + h * 512:kk + h * 512 + 512],
                            start=(kk == 0), stop=(kk == D_CONV - 1))
                    nc.scalar.activation(xcbf_sb[j][:, cs], pc[:], AF.Silu,
                                         bias=cb_sb[:, j, :])

                def mchain(m, h):
                    cs = slice(h * 512, (h + 1) * 512)
                    ps = psM.tile([128, 512], f32, tag="psM")
                    for k in range(NKT):
                        nc.tensor.matmul(
                            ps[:],
                            lhsT=win_sb[:, k, m * 128:(m + 1) * 128],
                            rhs=hTbf[:, k, cs],
                            start=(k == 0), stop=(k == NKT - 1))
                    return ps

                def x_half(h):
                    # x m-chains with lag-1 convs (j never stalls on its evac)
                    cs = slice(h * 512, (h + 1) * 512)
                    for m in range(4):
                        ps = mchain(m, h)
                        nc.vector.tensor_tensor(
                            out=x_sb[m][:, D_CONV - 1 + h * 512:D_CONV - 1 + (h + 1) * 512],
                            in0=ps[:], in1=inv_bc[:, cs], op=OP.mult)
                        if m >= 1:
                            conv_j(m - 1, h)

                def xproj_cc(h):
                    cs = slice(h * 512, (h + 1) * 512)
                    conv_j(3, h)
                    for k in range(NCT):
                        nc.tensor.matmul(
                            ps96[:, cs],
                            lhsT=wxp_sb[:, k, :],
                            rhs=xcbf_sb[k][:, cs],
                            start=(k == 0), stop=(k == NCT - 1))
                    xdbl_part = bpool.tile([96, 512], bf16, tag=f"xdblp{h}")
                    nc.scalar.activation(xdbl_part[:], ps96[:, cs], AF.Copy)
                    # h1 chain on the Pool queue (idle until the scan) so the
                    # two allreduce chains don't serialize on SP
                    deng = nc.sync if h == 0 else nc.gpsimd
                    deng.dma_start(out=cc_in[h], in_=xdbl_part[:])
                    if profile_mode:
                        deng.dma_start(out=cc_out[h], in_=cc_in[h])
                    else:
                        nc.gpsimd.collective_compute(
                            "AllReduce", mybir.AluOpType.add, replica_groups=RG,
                            ins=[cc_in[h]], outs=[cc_out[h]])
                    deng.dma_start(out=xdbl_sb[:, cs], in_=cc_out[h])

                def z_half(h):
                    # write scaled z into sz; the Silu runs later in one
                    # batch (between softplus Lns and dA Exps) so its table
                    # loads never interleave with the Exp/Ln block
                    cs = slice(h * 512, (h + 1) * 512)
                    for m in range(4, 8):
                        ps = mchain(m, h)
                        j = m - 4
                        nc.vector.tensor_tensor(out=sz_sb[:, j, cs], in0=ps[:],
                                                in1=inv_bc[:, cs], op=OP.mult)

                rms_tail(0)
                for i in range(4, 8):
                    do_square(i)
                rms_tail(1)
                _mark("rmsT1")
                x_half(0)
                for i in range(4, 8):
                    do_transpose(i)
                tps.close()  # free psT banks for psD
                xproj_cc(0)
                _mark("inproj0")
                x_half(1)
                xproj_cc(1)
                _mark("inproj1")
                if debug:
                    nc.sync.dma_start(out=dbg["d_ss"].ap(), in_=ss_all[:])

                _mark("xproj")
                psD = ptp.enter_context(tc.tile_pool(name="psD", bufs=2, space="PSUM"))
                spool = ptp.enter_context(tc.tile_pool(name="spool", bufs=2))
                p1p = ptp.enter_context(tc.tile_pool(name="p1p", bufs=1))

                # dt_proj + softplus: softplus(u+b) = ln(1+exp(u+b)).
                p1v = [p1p.tile([128, T], f32, tag=f"p1v{j}", name=f"p1v{j}")
                       for j in range(NCT)]
                for h in range(2):
                    cs = slice(h * 512, (h + 1) * 512)
                    for j in range(NCT):
                        psd = psD.tile([128, 512], f32, tag="psD")
                        nc.tensor.matmul(
                            psd[:],
                            lhsT=wdt_sb[:, j * 128:(j + 1) * 128],
                            rhs=xdbl_sb[0:DT_RANK, cs],
                            start=True, stop=True)
                        e1 = spool.tile([128, 512], f32, tag="sp_e")
                        nc.scalar.activation(e1[:], psd[:], AF.Exp,
                                             bias=dtb_sb[:, j, :])
                        nc.vector.tensor_scalar_add(p1v[j][:, cs], e1[:], 1.0)
                for j in range(NCT):
                    for h in range(2):
                        cs = slice(h * 512, (h + 1) * 512)
                        nc.scalar.activation(dt_sb[:, j, cs], p1v[j][:, cs], AF.Ln)
                    nc.vector.tensor_tensor(
                        out=dtxc_sb[j][:], in0=dt_sb[:, j, :],
                        in1=xcbf_sb[j][:], op=OP.mult)
                # z m-chains after the dt block: lower PE priority, so the
                # psd matmuls preempt them when each allreduce half returns
                z_half(0)
                z_half(1)

            _mark("dt")
            if debug:
                nc.sync.dma_start(out=dbg["d_invbc"].ap(), in_=inv_bc[:])
                nc.sync.dma_start(out=dbg["d_hT"].ap(), in_=hTbf[:])
                nc.sync.dma_start(out=dbg["d_x0"].ap(), in_=x_sb[0][:])
                nc.sync.dma_start(out=dbg["d_xdbl"].ap(), in_=xdbl_sb[:])
        # ===================== scan block (n-pair outer, j-inner) =========
        with ExitStack() as back:
            woutp = back.enter_context(tc.tile_pool(name="woutp", bufs=1))
            wout_sb = woutp.tile([128, NCT, D_MODEL], f32r, tag="wout")

            p6 = back.enter_context(ExitStack())
            bcp = p6.enter_context(tc.tile_pool(name="bcp", bufs=3))
            dAp = p6.enter_context(tc.tile_pool(name="dAp", bufs=3))
            dBup = p6.enter_context(tc.tile_pool(name="dBup", bufs=3))
            hp = p6.enter_context(tc.tile_pool(name="hp", bufs=4))
            prp = p6.enter_context(tc.tile_pool(name="prp", bufs=3))
            gat = p6.enter_context(tc.tile_pool(name="gat", bufs=2))
            psY = p6.enter_context(tc.tile_pool(name="psY", bufs=1, space="PSUM"))

            psy = [psY.tile([128, T], f32, tag=f"psy{j}", name=f"psy{j}")
                   for j in range(NCT)]

            # States processed in pairs: [128, 2T] tiles with a zeroed seam
            # column resetting the scan at the segment boundary.
            NP = D_STATE // 2
            NPRE = 2
            bc_tiles = {}

            def load_bc(p):
                t = bcp.tile([128, 2, 2 * T], bf16, tag="bc")
                # free layout per B/C: [n-pair(2) x t-half(2) x 512];
                # broadcast rows of cc_out[2,96,512] to 128 partitions
                for bc_i in range(2):
                    base = t[:, bc_i, :]
                    for h in range(2):
                        nc.sync.dma_start(
                            out=bass.AP(tensor=base.tensor,
                                        offset=base.offset + h * 512,
                                        ap=[base.ap[0], [T, 2], [1, 512]]),
                            in_=bass.AP(
                                tensor=cc_out,
                                offset=96 * 512 * h + (64 + 16 * bc_i + 2 * p) * 512,
                                ap=[[0, 128], [512, 2], [1, 512]]))
                bc_tiles[p] = t

            def rep2(ap):
                return bass.AP(tensor=ap.tensor, offset=ap.offset,
                               ap=[ap.ap[0], [0, 2], ap.ap[1]])

            for p in range(NPRE):
                load_bc(p)

            # Scans are DVE-only (codegen rejects TensorScalarPtr on Pool).
            # DVE: dBu + scan (+ a few prods); Pool: most prods (plain TT).
            pending = []  # (j, p, h_tile)
            n_pend = [0]

            def flush_pending():
                for (j, p, hsc) in pending:
                    prod = prp.tile([128, 2 * T], bf16, tag="prod")
                    idx = n_pend[0]; n_pend[0] += 1
                    peng = nc.vector if idx % 10 == 9 else nc.gpsimd
                    peng.tensor_tensor(out=prod[:], in0=hsc[:],
                                       in1=bc_tiles[p][:, 1, :], op=OP.mult)
                    for s in range(2):
                        n = 2 * p + s
                        for h in range(2):
                            nc.tensor.matmul(
                                psy[j][:, h * 512:(h + 1) * 512],
                                lhsT=identbf_sb[:],
                                rhs=prod[:, s * T + h * 512:s * T + (h + 1) * 512],
                                start=(n == 0), stop=(n == D_STATE - 1))
                pending.clear()

            for p in range(NP):
                if p + NPRE < NP:
                    load_bc(p + NPRE)
                if p == 3:
                    # deferred z-gates: one Silu batch, in place, emitted
                    # once all softplus Exp/Ln work has long retired so the
                    # scheduler can't interleave it into that table set
                    nc.scalar.activation(sz_sb[:], sz_sb[:], AF.Silu)
                if p == 2:
                    # out_proj weights after the early B/C prefetches
                    nc.sync.dma_start(
                        out=wout_sb[:],
                        in_=w_out_t.ap().rearrange("(k p) n -> p k n", p=128))
                for j in range(NCT):
                    dA = dAp.tile([128, 2 * T], bf16, tag="dA")
                    for s in range(2):
                        nc.scalar.activation(dA[:, s * T:(s + 1) * T],
                                             dt_sb[:, j, :], AF.Exp,
                                             scale=a_sb[:, j, 2 * p + s:2 * p + s + 1])
                    nc.vector.memset(dA[:, T:T + 1], 0.0)  # segment seam
                    dBu = dBup.tile([128, 2 * T], bf16, tag="dBu")
                    nc.vector.tensor_tensor(out=dBu[:], in0=rep2(dtxc_sb[j][:]),
                                            in1=bc_tiles[p][:, 0, :], op=OP.mult)
                    hsc = hp.tile([128, 2 * T], bf16, tag="h")
                    nc.vector.tensor_tensor_scan(
                        hsc[:], dA[:], dBu[:], 0.0, OP.mult, OP.add)
                    pending.append((j, p, hsc))
                flush_pending()

            first = True
            for h in range(2):
                cs = slice(h * 512, (h + 1) * 512)
                for j in range(NCT):
                    t1 = gat.tile([128, 512], f32, tag="t1")
                    nc.vector.scalar_tensor_tensor(
                        out=t1[:], in0=xcbf_sb[j][:, cs], scalar=d_sb[:, j, :],
                        in1=psy[j][:, cs], op0=OP.mult, op1=OP.add)
                    # y2 on Pool (free at scan end) halves the PE-idle gap
                    nc.gpsimd.tensor_tensor(out=y2_sb[j][:, cs], in0=t1[:],
                                            in1=sz_sb[:, j, cs], op=OP.mult)
                    if first:
                        # keep the PE p-state hot through the y2 window by
                        # recycling the consumed psy[0] bank as scratch
                        first = False
                        for w in range(16):
                            nc.tensor.matmul(
                                psy[0][:, 0:512], lhsT=identbf_sb[:],
                                rhs=dtxc_sb[0][:, 0:512],
                                start=True, stop=True)

            _mark("scan")
            if debug:
                nc.sync.dma_start(out=dbg["d_sz0"].ap(), in_=sz_sb[:, 0, :])
                nc.sync.dma_start(out=dbg["d_dt"].ap(), in_=dt_sb[:])
                nc.sync.dma_start(out=dbg["d_y20"].ap(), in_=y2_sb[0][:])
            p6.close()  # frees scan pools + psY banks before out_proj
            # ================== out_proj partial ======================
            with ExitStack() as p7:
                psO = p7.enter_context(tc.tile_pool(name="psO", bufs=4, space="PSUM"))
                oev = p7.enter_context(tc.tile_pool(name="oev", bufs=2))
                for mp in range(4):  # m-pairs
                    ot = oev.tile([128, 2, T], bf16, tag="oev")
                    for mi in range(2):
                        m = mp * 2 + mi
                        for h in range(2):
                            pso = psO.tile([128, 512], f32, tag="psO")
                            for k in range(NCT):
                                nc.tensor.matmul(
                                    pso[:],
                                    lhsT=wout_sb[:, k, m * 128:(m + 1) * 128],
                                    rhs=y2_sb[k][:, h * 512:(h + 1) * 512],
                                    start=(k == 0), stop=(k == NCT - 1))
                            nc.scalar.activation(
                                ot[:, mi, h * 512:(h + 1) * 512], pso[:], AF.Copy)
                    nc.sync.dma_start(
                        out=bass.AP(tensor=part_out, offset=mp * 2 * 128 * T,
                                    ap=[[T, 128], [128 * T, 2], [1, T]]),
                        in_=ot[:])

    _mark("out_proj")
    nc.compile()
    return nc


def _get_program():
    if "nc" not in _CACHE:
        _CACHE["nc"] = _build_program()
    return _CACHE["nc"]


def kernel(hidden_states, norm_weight, in_proj_w, conv_w, conv_b, x_proj_w,
           dt_proj_w, dt_proj_b, A_log, D, out_proj_w):
    from concourse.bass_utils import run_bass_kernel_spmd
    import ml_dtypes

    bf = ml_dtypes.bfloat16

    hidden_states = np.asarray(hidden_states, dtype=np.float32)
    norm_weight = np.asarray(norm_weight, dtype=np.float32)
    in_proj_w = np.asarray(in_proj_w, dtype=np.float32)
    conv_w = np.asarray(conv_w, dtype=np.float32)
    conv_b = np.asarray(conv_b, dtype=np.float32)
    x_proj_w = np.asarray(x_proj_w, dtype=np.float32)
    dt_proj_w = np.asarray(dt_proj_w, dtype=np.float32)
    dt_proj_b = np.asarray(dt_proj_b, dtype=np.float32)
    A_log = np.asarray(A_log, dtype=np.float32)
    D = np.asarray(D, dtype=np.float32)
    out_proj_w = np.asarray(out_proj_w, dtype=np.float32)

    nc = _get_program()

    a_neg_full = -np.exp(A_log)  # [2048, 16]
    ident_bf = np.eye(128, dtype=bf)
    eye128 = np.eye(128, dtype=np.float32)

    in_maps = []
    for c in range(8):
        b, j = c // 4, c % 4
        sl = slice(CH * j, CH * (j + 1))
        w_in_cat = np.concatenate(
            [in_proj_w[sl], in_proj_w[D_INNER + CH * j:D_INNER + CH * (j + 1)]],
            axis=0)  # [2CH, D_MODEL]
        w_in_fold = w_in_cat * norm_weight[None, :]
        cpack = np.concatenate(
            [a_neg_full[sl], conv_w[sl], conv_b[sl, None], dt_proj_b[sl, None],
             D[sl, None]], axis=1).astype(np.float32)
        # diag_cw[p, j*512 + kk*128 + c] = conv_w[j*128+p, kk] * (c == p)
        dk = conv_w[sl].reshape(NCT, 128, D_CONV)            # [j, p, kk]
        diag = np.einsum('jpk,pc->pjkc', dk, eye128)         # [p, j, kk, c]
        in_maps.append({
            "hid_bf": hidden_states[b].astype(bf),
            "w_in_bf": np.ascontiguousarray(w_in_fold.T).astype(bf),
            "w_xp_bf": np.ascontiguousarray(x_proj_w[:, sl].T).astype(bf),
            "w_dt_bf": np.ascontiguousarray(dt_proj_w[sl, :].T).astype(bf),
            "w_out_T": np.ascontiguousarray(out_proj_w[:, sl].T),
            "cpack": np.ascontiguousarray(cpack),
            "ident_bf": ident_bf,
            "diag_cw": np.ascontiguousarray(diag.reshape(128, -1)).astype(bf),
        })

    import os
    kw = {}
    if os.environ.get("MAMBA_TRACE"):
        kw = dict(trace=True, tmpdir=os.environ.get("MAMBA_TRACE_DIR") or None)
    res = run_bass_kernel_spmd(nc, in_maps, list(range(8)), **kw)
    _CACHE["last_results"] = res

    out = np.zeros((2, T, D_MODEL), np.float32)
    for c in range(8):
        b = c // 4
        out[b] += res.results[c]["part_out"].T.astype(np.float32)
    return out, hidden_states


# revision 85
# speedup vs baseline: 1.0489x; 1.0368x over previous
"""Mamba block (RMSNorm -> in_proj -> causal conv1d -> selective scan -> out_proj)
for Trainium2, SPMD over 8 NeuronCores.

Sharding: batch (2) x d_inner (2048 -> 4 slices of 512).
  core c: batch c//4, channels [512*(c%4), 512*(c%4)+512).
Each core computes its partial out_proj contribution; the host sums the 4
partials per batch and stacks batches.  A small on-device AllReduce (96x1024)
merges the x_proj partial sums across the 4 cores of each batch.

Engine plan (per core):
  PE   : transposes (bf16), in_proj/x_proj/dt_proj matmuls (bf16),
         scan y-accumulate via identity matmuls, out_proj (f32r).
  Act  : rms squares + ln/exp, psum evacs, silus, softplus, 64 dA exps.
  DVE  : rms eps/recip, x-evac scale, conv j0/j1, softplus +1, dtxc,
         dBu and prod multiplies (bf16 2x mode), 8 scans, y2.
  Pool : z-evac scale, conv j2/j3, 56 scans (scan runs at 0.6 eff vs
         TT-mult's 0.42, so Pool scans and DVE multiplies).
norm_weight is folded into W_in host-side; the 1/(rms+eps) per-token scale is
applied at in_proj PSUM evacuation via a broadcast tile (DRAM bounce).
"""

import math
import sys

import numpy as np

sys.path.insert(0, "/opt/trn_rl_repo")

D_MODEL = 1024
D_STATE = 16
D_CONV = 4
D_INNER = 2048
DT_RANK = 64  # ceil(1024/16)
EPS = 1e-5

T = 1024          # tokens per batch
CH = 512          # channels per core
NCT = CH // 128   # channel tiles per core (4)
NKT = D_MODEL // 128  # dmodel tiles (8)
N_DVE_SCAN = 2    # states per j whose scan runs on DVE (rest on Pool)

_CACHE = {}
_PHASE_MARKS = []


def _build_program(profile_mode=False, debug=False):
    from contextlib import ExitStack

    import concourse.bacc as bacc
    import concourse.bass as bass
    import concourse.tile as tile
    from concourse import mybir

    f32 = mybir.dt.float32
    f32r = mybir.dt.float32r
    bf16 = mybir.dt.bfloat16
    AF = mybir.ActivationFunctionType
    OP = mybir.AluOpType

    nc = bacc.Bacc("TRN2", target_bir_lowering=False, debug=False, num_devices=8)
    _PHASE_MARKS.clear()
    def _mark(p):
        _PHASE_MARKS.append((p, nc.next_id()))

    hid_t = nc.dram_tensor("hid_bf", [T, D_MODEL], bf16, kind="ExternalInput")
    w_in_t = nc.dram_tensor("w_in_bf", [D_MODEL, 2 * CH], bf16, kind="ExternalInput")
    w_xp_t = nc.dram_tensor("w_xp_bf", [CH, 96], bf16, kind="ExternalInput")
    w_dt_t = nc.dram_tensor("w_dt_bf", [DT_RANK, CH], bf16, kind="ExternalInput")
    w_out_t = nc.dram_tensor("w_out_T", [CH, D_MODEL], f32r, kind="ExternalInput")
    cpack_t = nc.dram_tensor("cpack", [CH, 23], f32, kind="ExternalInput")
    ident_bf = nc.dram_tensor("ident_bf", [128, 128], bf16, kind="ExternalInput")
    # conv taps as diagonal matrices: row p holds diag(conv_w[j*128+p, kk])
    # packed so tile[:, j, kk, :] is the lhsT for tap kk of channel tile j
    diag_cw = nc.dram_tensor("diag_cw", [128, NCT * D_CONV * 128], bf16,
                             kind="ExternalInput")

    part_out = nc.dram_tensor("part_out", [D_MODEL, T], bf16, kind="ExternalOutput")
    if debug:
        dbg = {
            "d_invbc": nc.dram_tensor("d_invbc", [128, T], f32, kind="ExternalOutput"),
            "d_hT": nc.dram_tensor("d_hT", [128, NKT, T], bf16, kind="ExternalOutput"),
            "d_x0": nc.dram_tensor("d_x0", [128, T + 3], f32, kind="ExternalOutput"),
            "d_xc0": nc.dram_tensor("d_xc0", [128, T], f32, kind="ExternalOutput"),
            "d_sz0": nc.dram_tensor("d_sz0", [128, T], f32, kind="ExternalOutput"),
            "d_xdbl": nc.dram_tensor("d_xdbl", [96, T], bf16, kind="ExternalOutput"),
            "d_dt": nc.dram_tensor("d_dt", [128, NCT, T], f32, kind="ExternalOutput"),
            "d_y20": nc.dram_tensor("d_y20", [128, T], f32r, kind="ExternalOutput"),
            "d_ss": nc.dram_tensor("d_ss", [128, NKT], f32, kind="ExternalOutput"),
        }

    inv_dram = nc.dram_tensor("inv_dram", [T], f32)
    cc_in = nc.dram_tensor("cc_in", [2, 96, 512], bf16)
    cc_out = nc.dram_tensor("cc_out", [2, 96, 512], bf16)

    RG = [[0, 1, 2, 3], [4, 5, 6, 7]]

    with tile.TileContext(nc) as tc, ExitStack() as ctx:
        consts = ctx.enter_context(tc.tile_pool(name="consts", bufs=1))
        persist = ctx.enter_context(tc.tile_pool(name="persist", bufs=1))

        # ---- constant + input loads (HWDGE is a single-slot resource:
        # emission order = dispatch order; identbf first for transposes) ----
        identbf_sb = consts.tile([128, 128], bf16, tag="identbf")
        nc.sync.dma_start(out=identbf_sb[:], in_=ident_bf.ap())

        # persistent activations (only those read by the scan/out_proj)
        xcbf_sb = [persist.tile([128, T], bf16, tag=f"xcb{j}", name=f"xcb{j}") for j in range(NCT)]
        sz_sb = persist.tile([128, NCT, T], f32, tag="sz")
        dt_sb = persist.tile([128, NCT, T], f32, tag="dt")
        dtxc_sb = [persist.tile([128, T], bf16, tag=f"dtxc{j}", name=f"dtxc{j}") for j in range(NCT)]
        y2_sb = [persist.tile([128, T], f32r, tag=f"y2{j}", name=f"y2{j}") for j in range(NCT)]

        _mark("consts")
        with ExitStack() as front:
            hidp = front.enter_context(tc.tile_pool(name="hidp", bufs=1))
            winp = front.enter_context(tc.tile_pool(name="winp", bufs=1))
            sst = front.enter_context(tc.tile_pool(name="sst", bufs=1))
            scr = front.enter_context(tc.tile_pool(name="scr", bufs=1))
            fper = front.enter_context(tc.tile_pool(name="fper", bufs=1))

            hTbf = fper.tile([128, NKT, T], bf16, tag="hT", name="hT")
            x_sb = [fper.tile([128, T + D_CONV - 1], bf16, tag=f"x{j}", name=f"x{j}")
                    for j in range(NCT)]
            diag_sb = fper.tile([128, NCT, D_CONV, 128], bf16, tag="diagcw")
            xdbl_sb = fper.tile([96, T], bf16, tag="xdbl")
            inv_bc = fper.tile([128, T], f32, tag="invbc")
            for j in range(NCT):
                nc.vector.memset(x_sb[j][:, 0:D_CONV - 1], 0.0)

            hid_sb = hidp.tile([128, NKT, T], bf16, tag="hid")
            win_sb = winp.tile([128, NKT, 2 * CH], bf16, tag="win")
            # hid halves [128,4,1024]; w_in split x-cols/z-cols so the h0
            # x-chains can start before the z columns arrive
            def hid_dma(hh):
                nc.sync.dma_start(
                    out=hid_sb[:, hh * 4:(hh + 1) * 4, :],
                    in_=bass.AP(tensor=hid_t, offset=hh * 4 * 128 * D_MODEL,
                                ap=[[D_MODEL, 128], [128 * D_MODEL, 4], [1, D_MODEL]]),
                )

            def win_dma(hh):
                nc.sync.dma_start(
                    out=win_sb[:, :, hh * 512:(hh + 1) * 512],
                    in_=bass.AP(tensor=w_in_t, offset=hh * 512,
                                ap=[[2 * CH, 128], [128 * 2 * CH, NKT], [1, 512]]))

            hid_dma(0)
            win_dma(0)
            cpk_sb = consts.tile([128, NCT, 23], f32, tag="cpk")
            nc.sync.dma_start(out=cpk_sb[:],
                              in_=cpack_t.ap().rearrange("(j p) n -> p j n", p=128))
            wxp_sb = consts.tile([128, NCT, 96], bf16, tag="wxp")
            nc.sync.dma_start(out=wxp_sb[:],
                              in_=w_xp_t.ap().rearrange("(j p) n -> p j n", p=128))
            wdt_sb = consts.tile([DT_RANK, CH], bf16, tag="wdt")
            nc.sync.dma_start(out=wdt_sb[:], in_=w_dt_t.ap())
            hid_dma(1)
            nc.sync.dma_start(out=diag_sb[:], in_=diag_cw.ap())
            win_dma(1)
            a_sb = cpk_sb[:, :, 0:16]
            cw_sb = cpk_sb[:, :, 16:20]
            cb_sb = cpk_sb[:, :, 20:21]
            dtb_sb = cpk_sb[:, :, 21:22]
            d_sb = cpk_sb[:, :, 22:23]

            # ---- rms stats + transposes (per hid t-tile i) ----
            scr_t = scr.tile([128, T], bf16, tag="scr")
            ss_all = sst.tile([128, NKT], f32, tag="ss")
            with ExitStack() as ptp:
                psM = ptp.enter_context(tc.tile_pool(name="psM", bufs=3, space="PSUM"))
                psC = ptp.enter_context(tc.tile_pool(name="psC", bufs=1, space="PSUM"))
                zt_p = ptp.enter_context(tc.tile_pool(name="ztp", bufs=2))
                psX = ptp.enter_context(tc.tile_pool(name="psX", bufs=1, space="PSUM"))
                bpool = ptp.enter_context(tc.tile_pool(name="bpool", bufs=2))
                ps96 = psX.tile([96, T], f32, tag="ps96")
                tps = ptp.enter_context(ExitStack())
                psT = tps.enter_context(tc.tile_pool(name="psT", bufs=2, space="PSUM"))

                def do_square(i):
                    nc.scalar.activation(scr_t[:], hid_sb[:, i, :], AF.Square,
                                         accum_out=ss_all[:, i:i + 1])

                def do_transpose(i):
                    for gh in range(2):
                        pst = psT.tile([128, 512], bf16, tag="psT")
                        for q in range(4):
                            g = gh * 4 + q
                            nc.tensor.transpose(
                                pst[:, q * 128:(q + 1) * 128],
                                hid_sb[:, i, g * 128:(g + 1) * 128], identbf_sb[:])
                        # DVE copy (bf16 2x): Act is busy with the squares
                        nc.vector.tensor_copy(
                            out=hTbf[:, gh * 4:(gh + 1) * 4, i * 128:(i + 1) * 128],
                            in_=pst[:])

                for i in range(4):
                    do_square(i)
                    do_transpose(i)

                _mark("rmsT0")
                # rms tail per t-half: inv for t in half h needs only hid
                # tiles h*4..h*4+3 (keeps the half-pipelined in_proj legal).
                # 1/(rms+eps) ~= rsqrt(ms) to ~1e-5 rel; single Act op whose
                # table set also holds Square (no reload between them).
                def rms_tail(h):
                    hs = slice(h * 4, (h + 1) * 4)
                    den = sst.tile([128, 4], f32, tag=f"den{h}")
                    nc.scalar.activation(den[:], ss_all[:, hs], AF.Sqrt,
                                         scale=1.0 / D_MODEL)
                    den2 = sst.tile([128, 4], f32, tag=f"den2{h}")
                    nc.vector.tensor_scalar_add(den2[:], den[:], EPS)
                    inv = sst.tile([128, 4], f32, tag=f"inv{h}")
                    nc.vector.reciprocal(inv[:], den2[:])
                    nc.sync.dma_start(
                        out=bass.AP(tensor=inv_dram, offset=h * 512,
                                    ap=[[1, 128], [128, 4]]),
                        in_=inv[:])
                    nc.sync.dma_start(
                        out=inv_bc[:, h * 512:(h + 1) * 512],
                        in_=bass.AP(tensor=inv_dram, offset=h * 512,
                                    ap=[[0, 128], [1, 512]]))

                def conv_j(j, h):
                    # causal depthwise conv as 4 PE diag-matmuls into PSUM
                    cs = slice(h * 512, (h + 1) * 512)
                    pc = psC.tile([128, 512], f32, tag="psC")
                    for kk in range(D_CONV):
                        nc.tensor.matmul(
                            pc[:],
                            lhsT=diag_sb[:, j, kk, :],
                            rhs=x_sb[j][:, kk + h * 512:kk + h * 512 + 512],
                            start=(kk == 0), stop=(kk == D_CONV - 1))
                    nc.scalar.activation(xcbf_sb[j][:, cs], pc[:], AF.Silu,
                                         bias=cb_sb[:, j, :])

                def mchain(m, h):
                    cs = slice(h * 512, (h + 1) * 512)
                    ps = psM.tile([128, 512], f32, tag="psM")
                    for k in range(NKT):
                        nc.tensor.matmul(
                            ps[:],
                            lhsT=win_sb[:, k, m * 128:(m + 1) * 128],
                            rhs=hTbf[:, k, cs],
                            start=(k == 0), stop=(k == NKT - 1))
                    return ps

                def x_half(h):
                    # x m-chains with lag-1 convs (j never stalls on its evac)
                    cs = slice(h * 512, (h + 1) * 512)
                    for m in range(4):
                        ps = mchain(m, h)
                        nc.vector.tensor_tensor(
                            out=x_sb[m][:, D_CONV - 1 + h * 512:D_CONV - 1 + (h + 1) * 512],
                            in0=ps[:], in1=inv_bc[:, cs], op=OP.mult)
                        if m >= 1:
                            conv_j(m - 1, h)

                def xproj_cc(h):
                    cs = slice(h * 512, (h + 1) * 512)
                    conv_j(3, h)
                    for k in range(NCT):
                        nc.tensor.matmul(
                            ps96[:, cs],
                            lhsT=wxp_sb[:, k, :],
                            rhs=xcbf_sb[k][:, cs],
                            start=(k == 0), stop=(k == NCT - 1))
                    xdbl_part = bpool.tile([96, 512], bf16, tag=f"xdblp{h}")
                    nc.scalar.activation(xdbl_part[:], ps96[:, cs], AF.Copy)
                    # h1 chain on the Pool queue (idle until the scan) so the
                    # two allreduce chains don't serialize on SP
                    deng = nc.sync if h == 0 else nc.gpsimd
                    deng.dma_start(out=cc_in[h], in_=xdbl_part[:])
                    if profile_mode:
                        deng.dma_start(out=cc_out[h], in_=cc_in[h])
                    else:
                        nc.gpsimd.collective_compute(
                            "AllReduce", mybir.AluOpType.add, replica_groups=RG,
                            ins=[cc_in[h]], outs=[cc_out[h]])
                    deng.dma_start(out=xdbl_sb[:, cs], in_=cc_out[h])

                def z_half(h):
                    # write scaled z into sz; the Silu runs later in one
                    # batch (between softplus Lns and dA Exps) so its table
                    # loads never interleave with the Exp/Ln block
                    cs = slice(h * 512, (h + 1) * 512)
                    for m in range(4, 8):
                        ps = mchain(m, h)
                        j = m - 4
                        nc.vector.tensor_tensor(out=sz_sb[:, j, cs], in0=ps[:],
                                                in1=inv_bc[:, cs], op=OP.mult)

                rms_tail(0)
                for i in range(4, 8):
                    do_square(i)
                rms_tail(1)
                _mark("rmsT1")
                x_half(0)
                for i in range(4, 8):
                    do_transpose(i)
                tps.close()  # free psT banks for psD
                xproj_cc(0)
                _mark("inproj0")
                x_half(1)
                xproj_cc(1)
                _mark("inproj1")
                if debug:
                    nc.sync.dma_start(out=dbg["d_ss"].ap(), in_=ss_all[:])

                _mark("xproj")
                psD = ptp.enter_context(tc.tile_pool(name="psD", bufs=2, space="PSUM"))
                spool = ptp.enter_context(tc.tile_pool(name="spool", bufs=2))
                p1p = ptp.enter_context(tc.tile_pool(name="p1p", bufs=1))

                # dt_proj + softplus: softplus(u+b) = ln(1+exp(u+b)).
                p1v = [p1p.tile([128, T], f32, tag=f"p1v{j}", name=f"p1v{j}")
                       for j in range(NCT)]
                for h in range(2):
                    cs = slice(h * 512, (h + 1) * 512)
                    for j in range(NCT):
                        psd = psD.tile([128, 512], f32, tag="psD")
                        nc.tensor.matmul(
                            psd[:],
                            lhsT=wdt_sb[:, j * 128:(j + 1) * 128],
                            rhs=xdbl_sb[0:DT_RANK, cs],
                            start=True, stop=True)
                        e1 = spool.tile([128, 512], f32, tag="sp_e")
                        nc.scalar.activation(e1[:], psd[:], AF.Exp,
                                             bias=dtb_sb[:, j, :])
                        nc.vector.tensor_scalar_add(p1v[j][:, cs], e1[:], 1.0)
                for j in range(NCT):
                    for h in range(2):
                        cs = slice(h * 512, (h + 1) * 512)
                        nc.scalar.activation(dt_sb[:, j, cs], p1v[j][:, cs], AF.Ln)
                    nc.vector.tensor_tensor(
                        out=dtxc_sb[j][:], in0=dt_sb[:, j, :],
                        in1=xcbf_sb[j][:], op=OP.mult)
                # z m-chains after the dt block: lower PE priority, so the
                # psd matmuls preempt them when each allreduce half returns
                z_half(0)
                z_half(1)

            _mark("dt")
            if debug:
                nc.sync.dma_start(out=dbg["d_invbc"].ap(), in_=inv_bc[:])
                nc.sync.dma_start(out=dbg["d_hT"].ap(), in_=hTbf[:])
                nc.sync.dma_start(out=dbg["d_x0"].ap(), in_=x_sb[0][:])
                nc.sync.dma_start(out=dbg["d_xdbl"].ap(), in_=xdbl_sb[:])
        # ===================== scan block (n-pair outer, j-inner) =========
        with ExitStack() as back:
            woutp = back.enter_context(tc.tile_pool(name="woutp", bufs=1))
            wout_sb = woutp.tile([128, NCT, D_MODEL], f32r, tag="wout")

            p6 = back.enter_context(ExitStack())
            bcp = p6.enter_context(tc.tile_pool(name="bcp", bufs=3))
            dAp = p6.enter_context(tc.tile_pool(name="dAp", bufs=3))
            dBup = p6.enter_context(tc.tile_pool(name="dBup", bufs=3))
            hp = p6.enter_context(tc.tile_pool(name="hp", bufs=4))
            prp = p6.enter_context(tc.tile_pool(name="prp", bufs=3))
            gat = p6.enter_context(tc.tile_pool(name="gat", bufs=2))
            psY = p6.enter_context(tc.tile_pool(name="psY", bufs=1, space="PSUM"))

            psy = [psY.tile([128, T], f32, tag=f"psy{j}", name=f"psy{j}")
                   for j in range(NCT)]

            # States processed in pairs: [128, 2T] tiles with a zeroed seam
            # column resetting the scan at the segment boundary.
            NP = D_STATE // 2
            NPRE = 2
            bc_tiles = {}

            def load_bc(p):
                t = bcp.tile([128, 2, 2 * T], bf16, tag="bc")
                # free layout per B/C: [n-pair(2) x t-half(2) x 512];
                # broadcast rows of cc_out[2,96,512] to 128 partitions
                for bc_i in range(2):
                    base = t[:, bc_i, :]
                    for h in range(2):
                        nc.sync.dma_start(
                            out=bass.AP(tensor=base.tensor,
                                        offset=base.offset + h * 512,
                                        ap=[base.ap[0], [T, 2], [1, 512]]),
                            in_=bass.AP(
                                tensor=cc_out,
                                offset=96 * 512 * h + (64 + 16 * bc_i + 2 * p) * 512,
                                ap=[[0, 128], [512, 2], [1, 512]]))
                bc_tiles[p] = t

            def rep2(ap):
                return bass.AP(tensor=ap.tensor, offset=ap.offset,
                               ap=[ap.ap[0], [0, 2], ap.ap[1]])

            for p in range(NPRE):
                load_bc(p)

            # Scans are DVE-only (codegen rejects TensorScalarPtr on Pool).
            # DVE: dBu + scan (+ a few prods); Pool: most prods (plain TT).
            pending = []  # (j, p, h_tile)
            n_pend = [0]

            def flush_pending():
                for (j, p, hsc) in pending:
                    prod = prp.tile([128, 2 * T], bf16, tag="prod")
                    idx = n_pend[0]; n_pend[0] += 1
                    peng = nc.vector if idx % 6 == 5 else nc.gpsimd
                    peng.tensor_tensor(out=prod[:], in0=hsc[:],
                                       in1=bc_tiles[p][:, 1, :], op=OP.mult)
                    for s in range(2):
                        n = 2 * p + s
                        for h in range(2):
                            nc.tensor.matmul(
                                psy[j][:, h * 512:(h + 1) * 512],
                                lhsT=identbf_sb[:],
                                rhs=prod[:, s * T + h * 512:s * T + (h + 1) * 512],
                                start=(n == 0), stop=(n == D_STATE - 1))
                pending.clear()

            for p in range(NP):
                if p + NPRE < NP:
                    load_bc(p + NPRE)
                if p == 3:
                    # deferred z-gates: one Silu batch, in place, emitted
                    # once all softplus Exp/Ln work has long retired so the
                    # scheduler can't interleave it into that table set
                    nc.scalar.activation(sz_sb[:], sz_sb[:], AF.Silu)
                if p == 2:
                    # out_proj weights after the early B/C prefetches
                    nc.sync.dma_start(
                        out=wout_sb[:],
                        in_=w_out_t.ap().rearrange("(k p) n -> p k n", p=128))
                for j in range(NCT):
                    dA = dAp.tile([128, 2 * T], bf16, tag="dA")
                    for s in range(2):
                        nc.scalar.activation(dA[:, s * T:(s + 1) * T],
                                             dt_sb[:, j, :], AF.Exp,
                                             scale=a_sb[:, j, 2 * p + s:2 * p + s + 1])
                    nc.vector.memset(dA[:, T:T + 1], 0.0)  # segment seam
                    dBu = dBup.tile([128, 2 * T], bf16, tag="dBu")
                    nc.vector.tensor_tensor(out=dBu[:], in0=rep2(dtxc_sb[j][:]),
                                            in1=bc_tiles[p][:, 0, :], op=OP.mult)
                    hsc = hp.tile([128, 2 * T], bf16, tag="h")
                    nc.vector.tensor_tensor_scan(
                        hsc[:], dA[:], dBu[:], 0.0, OP.mult, OP.add)
                    pending.append((j, p, hsc))
                flush_pending()

            first = True
            for h in range(2):
                cs = slice(h * 512, (h + 1) * 512)
                for j in range(NCT):
                    t1 = gat.tile([128, 512], f32, tag="t1")
                    nc.vector.scalar_tensor_tensor(
                        out=t1[:], in0=xcbf_sb[j][:, cs], scalar=d_sb[:, j, :],
                        in1=psy[j][:, cs], op0=OP.mult, op1=OP.add)
                    # y2 on Pool (free at scan end) halves the PE-idle gap
                    nc.gpsimd.tensor_tensor(out=y2_sb[j][:, cs], in0=t1[:],
                                            in1=sz_sb[:, j, cs], op=OP.mult)
                    if first:
                        # keep the PE p-state hot through the y2 window by
                        # recycling the consumed psy[0] bank as scratch
                        first = False
                        for w in range(16):
                            nc.tensor.matmul(
                                psy[0][:, 0:512], lhsT=identbf_sb[:],
                                rhs=dtxc_sb[0][:, 0:512],
                                start=True, stop=True)

            _mark("scan")
            if debug:
                nc.sync.dma_start(out=dbg["d_sz0"].ap(), in_=sz_sb[:, 0, :])
                nc.sync.dma_start(out=dbg["d_dt"].ap(), in_=dt_sb[:])
                nc.sync.dma_start(out=dbg["d_y20"].ap(), in_=y2_sb[0][:])
            p6.close()  # frees scan pools + psY banks before out_proj
            # ================== out_proj partial ======================
            with ExitStack() as p7:
                psO = p7.enter_context(tc.tile_pool(name="psO", bufs=4, space="PSUM"))
                oev = p7.enter_context(tc.tile_pool(name="oev", bufs=2))
                for mp in range(4):  # m-pairs
                    ot = oev.tile([128, 2, T], bf16, tag="oev")
                    for mi in range(2):
                        m = mp * 2 + mi
                        for h in range(2):
                            pso = psO.tile([128, 512], f32, tag="psO")
                            for k in range(NCT):
                                nc.tensor.matmul(
                                    pso[:],
                                    lhsT=wout_sb[:, k, m * 128:(m + 1) * 128],
                                    rhs=y2_sb[k][:, h * 512:(h + 1) * 512],
                                    start=(k == 0), stop=(k == NCT - 1))
                            nc.scalar.activation(
                                ot[:, mi, h * 512:(h + 1) * 512], pso[:], AF.Copy)
                    nc.sync.dma_start(
                        out=bass.AP(tensor=part_out, offset=mp * 2 * 128 * T,
                                    ap=[[T, 128], [128 * T, 2], [1, T]]),
                        in_=ot[:])

    _mark("out_proj")
    nc.compile()
    return nc


def _get_program():
    if "nc" not in _CACHE:
        _CACHE["nc"] = _build_program()
    return _CACHE["nc"]


def kernel(hidden_states, norm_weight, in_proj_w, conv_w, conv_b, x_proj_w,
           dt_proj_w, dt_proj_b, A_log, D, out_proj_w):
    from concourse.bass_utils import run_bass_kernel_spmd
    import ml_dtypes

    bf = ml_dtypes.bfloat16

    hidden_states = np.asarray(hidden_states, dtype=np.float32)
    norm_weight = np.asarray(norm_weight, dtype=np.float32)
    in_proj_w = np.asarray(in_proj_w, dtype=np.float32)
    conv_w = np.asarray(conv_w, dtype=np.float32)
    conv_b = np.asarray(conv_b, dtype=np.float32)
    x_proj_w = np.asarray(x_proj_w, dtype=np.float32)
    dt_proj_w = np.asarray(dt_proj_w, dtype=np.float32)
    dt_proj_b = np.asarray(dt_proj_b, dtype=np.float32)
    A_log = np.asarray(A_log, dtype=np.float32)
    D = np.asarray(D, dtype=np.float32)
    out_proj_w = np.asarray(out_proj_w, dtype=np.float32)

    nc = _get_program()

    a_neg_full = -np.exp(A_log)  # [2048, 16]
    ident_bf = np.eye(128, dtype=bf)
    eye128 = np.eye(128, dtype=np.float32)

    in_maps = []
    for c in range(8):
        b, j = c // 4, c % 4
        sl = slice(CH * j, CH * (j + 1))
        w_in_cat = np.concatenate(
            [in_proj_w[sl], in_proj_w[D_INNER + CH * j:D_INNER + CH * (j + 1)]],
            axis=0)  # [2CH, D_MODEL]
        w_in_fold = w_in_cat * norm_weight[None, :]
        cpack = np.concatenate(
            [a_neg_full[sl], conv_w[sl], conv_b[sl, None], dt_proj_b[sl, None],
             D[sl, None]], axis=1).astype(np.float32)
        # diag_cw[p, j*512 + kk*128 + c] = conv_w[j*128+p, kk] * (c == p)
        dk = conv_w[sl].reshape(NCT, 128, D_CONV)            # [j, p, kk]
        diag = np.einsum('jpk,pc->pjkc', dk, eye128)         # [p, j, kk, c]
        in_maps.append({
            "hid_bf": hidden_states[b].astype(bf),
            "w_in_bf": np.ascontiguousarray(w_in_fold.T).astype(bf),
            "w_xp_bf": np.ascontiguousarray(x_proj_w[:, sl].T).astype(bf),
            "w_dt_bf": np.ascontiguousarray(dt_proj_w[sl, :].T).astype(bf),
            "w_out_T": np.ascontiguousarray(out_proj_w[:, sl].T),
            "cpack": np.ascontiguousarray(cpack),
            "ident_bf": ident_bf,
            "diag_cw": np.ascontiguousarray(diag.reshape(128, -1)).astype(bf),
        })

    import os
    kw = {}
    if os.environ.get("MAMBA_TRACE"):
        kw = dict(trace=True, tmpdir=os.environ.get("MAMBA_TRACE_DIR") or None)
    res = run_bass_kernel_spmd(nc, in_maps, list(range(8)), **kw)
    _CACHE["last_results"] = res

    out = np.zeros((2, T, D_MODEL), np.float32)
    for c in range(8):
        b = c // 4
        out[b] += res.results[c]["part_out"].T.astype(np.float32)
    return out, hidden_states


# revision 86
# speedup vs baseline: 1.0508x; 1.0017x over previous
"""Mamba block (RMSNorm -> in_proj -> causal conv1d -> selective scan -> out_proj)
for Trainium2, SPMD over 8 NeuronCores.

Sharding: batch (2) x d_inner (2048 -> 4 slices of 512).
  core c: batch c//4, channels [512*(c%4), 512*(c%4)+512).
Each core computes its partial out_proj contribution; the host sums the 4
partials per batch and stacks batches.  A small on-device AllReduce (96x1024)
merges the x_proj partial sums across the 4 cores of each batch.

Engine plan (per core):
  PE   : transposes (bf16), in_proj/x_proj/dt_proj matmuls (bf16),
         scan y-accumulate via identity matmuls, out_proj (f32r).
  Act  : rms squares + ln/exp, psum evacs, silus, softplus, 64 dA exps.
  DVE  : rms eps/recip, x-evac scale, conv j0/j1, softplus +1, dtxc,
         dBu and prod multiplies (bf16 2x mode), 8 scans, y2.
  Pool : z-evac scale, conv j2/j3, 56 scans (scan runs at 0.6 eff vs
         TT-mult's 0.42, so Pool scans and DVE multiplies).
norm_weight is folded into W_in host-side; the 1/(rms+eps) per-token scale is
applied at in_proj PSUM evacuation via a broadcast tile (DRAM bounce).
"""

import math
import sys

import numpy as np

sys.path.insert(0, "/opt/trn_rl_repo")

D_MODEL = 1024
D_STATE = 16
D_CONV = 4
D_INNER = 2048
DT_RANK = 64  # ceil(1024/16)
EPS = 1e-5

T = 1024          # tokens per batch
CH = 512          # channels per core
NCT = CH // 128   # channel tiles per core (4)
NKT = D_MODEL // 128  # dmodel tiles (8)
N_DVE_SCAN = 2    # states per j whose scan runs on DVE (rest on Pool)

_CACHE = {}
_PHASE_MARKS = []


def _build_program(profile_mode=False, debug=False):
    from contextlib import ExitStack

    import concourse.bacc as bacc
    import concourse.bass as bass
    import concourse.tile as tile
    from concourse import mybir

    f32 = mybir.dt.float32
    f32r = mybir.dt.float32r
    bf16 = mybir.dt.bfloat16
    AF = mybir.ActivationFunctionType
    OP = mybir.AluOpType

    nc = bacc.Bacc("TRN2", target_bir_lowering=False, debug=False, num_devices=8)
    _PHASE_MARKS.clear()
    def _mark(p):
        _PHASE_MARKS.append((p, nc.next_id()))

    hid_t = nc.dram_tensor("hid_bf", [T, D_MODEL], bf16, kind="ExternalInput")
    w_in_t = nc.dram_tensor("w_in_bf", [D_MODEL, 2 * CH], bf16, kind="ExternalInput")
    w_xp_t = nc.dram_tensor("w_xp_bf", [CH, 96], bf16, kind="ExternalInput")
    w_dt_t = nc.dram_tensor("w_dt_bf", [DT_RANK, CH], bf16, kind="ExternalInput")
    w_out_t = nc.dram_tensor("w_out_T", [CH, D_MODEL], f32r, kind="ExternalInput")
    cpack_t = nc.dram_tensor("cpack", [CH, 23], f32, kind="ExternalInput")
    ident_bf = nc.dram_tensor("ident_bf", [128, 128], bf16, kind="ExternalInput")
    # conv taps as diagonal matrices: row p holds diag(conv_w[j*128+p, kk])
    # packed so tile[:, j, kk, :] is the lhsT for tap kk of channel tile j
    diag_cw = nc.dram_tensor("diag_cw", [128, NCT * D_CONV * 128], bf16,
                             kind="ExternalInput")

    part_out = nc.dram_tensor("part_out", [D_MODEL, T], bf16, kind="ExternalOutput")
    if debug:
        dbg = {
            "d_invbc": nc.dram_tensor("d_invbc", [128, T], f32, kind="ExternalOutput"),
            "d_hT": nc.dram_tensor("d_hT", [128, NKT, T], bf16, kind="ExternalOutput"),
            "d_x0": nc.dram_tensor("d_x0", [128, T + 3], f32, kind="ExternalOutput"),
            "d_xc0": nc.dram_tensor("d_xc0", [128, T], f32, kind="ExternalOutput"),
            "d_sz0": nc.dram_tensor("d_sz0", [128, T], f32, kind="ExternalOutput"),
            "d_xdbl": nc.dram_tensor("d_xdbl", [96, T], bf16, kind="ExternalOutput"),
            "d_dt": nc.dram_tensor("d_dt", [128, NCT, T], f32, kind="ExternalOutput"),
            "d_y20": nc.dram_tensor("d_y20", [128, T], f32r, kind="ExternalOutput"),
            "d_ss": nc.dram_tensor("d_ss", [128, NKT], f32, kind="ExternalOutput"),
        }

    inv_dram = nc.dram_tensor("inv_dram", [T], f32)
    cc_in = nc.dram_tensor("cc_in", [2, 96, 512], bf16)
    cc_out = nc.dram_tensor("cc_out", [2, 96, 512], bf16)

    RG = [[0, 1, 2, 3], [4, 5, 6, 7]]

    with tile.TileContext(nc) as tc, ExitStack() as ctx:
        consts = ctx.enter_context(tc.tile_pool(name="consts", bufs=1))
        persist = ctx.enter_context(tc.tile_pool(name="persist", bufs=1))

        # ---- constant + input loads (HWDGE is a single-slot resource:
        # emission order = dispatch order; identbf first for transposes) ----
        identbf_sb = consts.tile([128, 128], bf16, tag="identbf")
        nc.sync.dma_start(out=identbf_sb[:], in_=ident_bf.ap())

        # persistent activations (only those read by the scan/out_proj)
        xcbf_sb = [persist.tile([128, T], bf16, tag=f"xcb{j}", name=f"xcb{j}") for j in range(NCT)]
        sz_sb = persist.tile([128, NCT, T], f32, tag="sz")
        dt_sb = persist.tile([128, NCT, T], f32, tag="dt")
        dtxc_sb = [persist.tile([128, T], bf16, tag=f"dtxc{j}", name=f"dtxc{j}") for j in range(NCT)]
        y2_sb = [persist.tile([128, T], f32r, tag=f"y2{j}", name=f"y2{j}") for j in range(NCT)]

        _mark("consts")
        with ExitStack() as front:
            hidp = front.enter_context(tc.tile_pool(name="hidp", bufs=1))
            winp = front.enter_context(tc.tile_pool(name="winp", bufs=1))
            sst = front.enter_context(tc.tile_pool(name="sst", bufs=1))
            scr = front.enter_context(tc.tile_pool(name="scr", bufs=1))
            fper = front.enter_context(tc.tile_pool(name="fper", bufs=1))

            hTbf = fper.tile([128, NKT, T], bf16, tag="hT", name="hT")
            x_sb = [fper.tile([128, T + D_CONV - 1], bf16, tag=f"x{j}", name=f"x{j}")
                    for j in range(NCT)]
            diag_sb = fper.tile([128, NCT, D_CONV, 128], bf16, tag="diagcw")
            xdbl_sb = fper.tile([96, T], bf16, tag="xdbl")
            inv_bc = fper.tile([128, T], f32, tag="invbc")
            for j in range(NCT):
                nc.vector.memset(x_sb[j][:, 0:D_CONV - 1], 0.0)

            hid_sb = hidp.tile([128, NKT, T], bf16, tag="hid")
            win_sb = winp.tile([128, NKT, 2 * CH], bf16, tag="win")
            # hid halves [128,4,1024]; w_in split x-cols/z-cols so the h0
            # x-chains can start before the z columns arrive
            def hid_dma(hh):
                nc.sync.dma_start(
                    out=hid_sb[:, hh * 4:(hh + 1) * 4, :],
                    in_=bass.AP(tensor=hid_t, offset=hh * 4 * 128 * D_MODEL,
                                ap=[[D_MODEL, 128], [128 * D_MODEL, 4], [1, D_MODEL]]),
                )

            def win_dma(hh):
                nc.sync.dma_start(
                    out=win_sb[:, :, hh * 512:(hh + 1) * 512],
                    in_=bass.AP(tensor=w_in_t, offset=hh * 512,
                                ap=[[2 * CH, 128], [128 * 2 * CH, NKT], [1, 512]]))

            hid_dma(0)
            win_dma(0)
            cpk_sb = consts.tile([128, NCT, 23], f32, tag="cpk")
            nc.sync.dma_start(out=cpk_sb[:],
                              in_=cpack_t.ap().rearrange("(j p) n -> p j n", p=128))
            wxp_sb = consts.tile([128, NCT, 96], bf16, tag="wxp")
            nc.sync.dma_start(out=wxp_sb[:],
                              in_=w_xp_t.ap().rearrange("(j p) n -> p j n", p=128))
            wdt_sb = consts.tile([DT_RANK, CH], bf16, tag="wdt")
            nc.sync.dma_start(out=wdt_sb[:], in_=w_dt_t.ap())
            hid_dma(1)
            nc.sync.dma_start(out=diag_sb[:], in_=diag_cw.ap())
            win_dma(1)
            a_sb = cpk_sb[:, :, 0:16]
            cw_sb = cpk_sb[:, :, 16:20]
            cb_sb = cpk_sb[:, :, 20:21]
            dtb_sb = cpk_sb[:, :, 21:22]
            d_sb = cpk_sb[:, :, 22:23]

            # ---- rms stats + transposes (per hid t-tile i) ----
            scr_t = scr.tile([128, T], bf16, tag="scr")
            ss_all = sst.tile([128, NKT], f32, tag="ss")
            with ExitStack() as ptp:
                psM = ptp.enter_context(tc.tile_pool(name="psM", bufs=3, space="PSUM"))
                psC = ptp.enter_context(tc.tile_pool(name="psC", bufs=1, space="PSUM"))
                zt_p = ptp.enter_context(tc.tile_pool(name="ztp", bufs=2))
                psX = ptp.enter_context(tc.tile_pool(name="psX", bufs=1, space="PSUM"))
                bpool = ptp.enter_context(tc.tile_pool(name="bpool", bufs=2))
                ps96 = psX.tile([96, T], f32, tag="ps96")
                tps = ptp.enter_context(ExitStack())
                psT = tps.enter_context(tc.tile_pool(name="psT", bufs=2, space="PSUM"))

                def do_square(i):
                    nc.scalar.activation(scr_t[:], hid_sb[:, i, :], AF.Square,
                                         accum_out=ss_all[:, i:i + 1])

                def do_transpose(i):
                    for gh in range(2):
                        pst = psT.tile([128, 512], bf16, tag="psT")
                        for q in range(4):
                            g = gh * 4 + q
                            nc.tensor.transpose(
                                pst[:, q * 128:(q + 1) * 128],
                                hid_sb[:, i, g * 128:(g + 1) * 128], identbf_sb[:])
                        # DVE copy (bf16 2x): Act is busy with the squares
                        nc.vector.tensor_copy(
                            out=hTbf[:, gh * 4:(gh + 1) * 4, i * 128:(i + 1) * 128],
                            in_=pst[:])

                for i in range(4):
                    do_square(i)
                    do_transpose(i)

                _mark("rmsT0")
                # rms tail per t-half: inv for t in half h needs only hid
                # tiles h*4..h*4+3 (keeps the half-pipelined in_proj legal).
                # 1/(rms+eps) ~= rsqrt(ms) to ~1e-5 rel; single Act op whose
                # table set also holds Square (no reload between them).
                def rms_tail(h):
                    hs = slice(h * 4, (h + 1) * 4)
                    den = sst.tile([128, 4], f32, tag=f"den{h}")
                    nc.scalar.activation(den[:], ss_all[:, hs], AF.Sqrt,
                                         scale=1.0 / D_MODEL)
                    den2 = sst.tile([128, 4], f32, tag=f"den2{h}")
                    nc.vector.tensor_scalar_add(den2[:], den[:], EPS)
                    inv = sst.tile([128, 4], f32, tag=f"inv{h}")
                    nc.vector.reciprocal(inv[:], den2[:])
                    nc.sync.dma_start(
                        out=bass.AP(tensor=inv_dram, offset=h * 512,
                                    ap=[[1, 128], [128, 4]]),
                        in_=inv[:])
                    nc.sync.dma_start(
                        out=inv_bc[:, h * 512:(h + 1) * 512],
                        in_=bass.AP(tensor=inv_dram, offset=h * 512,
                                    ap=[[0, 128], [1, 512]]))

                def conv_j(j, h):
                    # causal depthwise conv as 4 PE diag-matmuls into PSUM
                    cs = slice(h * 512, (h + 1) * 512)
                    pc = psC.tile([128, 512], f32, tag="psC")
                    for kk in range(D_CONV):
                        nc.tensor.matmul(
                            pc[:],
                            lhsT=diag_sb[:, j, kk, :],
                            rhs=x_sb[j][:, kk + h * 512:kk + h * 512 + 512],
                            start=(kk == 0), stop=(kk == D_CONV - 1))
                    nc.scalar.activation(xcbf_sb[j][:, cs], pc[:], AF.Silu,
                                         bias=cb_sb[:, j, :])

                def mchain(m, h):
                    cs = slice(h * 512, (h + 1) * 512)
                    ps = psM.tile([128, 512], f32, tag="psM")
                    for k in range(NKT):
                        nc.tensor.matmul(
                            ps[:],
                            lhsT=win_sb[:, k, m * 128:(m + 1) * 128],
                            rhs=hTbf[:, k, cs],
                            start=(k == 0), stop=(k == NKT - 1))
                    return ps

                def x_half(h):
                    # x m-chains with lag-1 convs (j never stalls on its evac)
                    cs = slice(h * 512, (h + 1) * 512)
                    for m in range(4):
                        ps = mchain(m, h)
                        nc.vector.tensor_tensor(
                            out=x_sb[m][:, D_CONV - 1 + h * 512:D_CONV - 1 + (h + 1) * 512],
                            in0=ps[:], in1=inv_bc[:, cs], op=OP.mult)
                        if m >= 1:
                            conv_j(m - 1, h)

                def xproj_cc(h):
                    cs = slice(h * 512, (h + 1) * 512)
                    conv_j(3, h)
                    for k in range(NCT):
                        nc.tensor.matmul(
                            ps96[:, cs],
                            lhsT=wxp_sb[:, k, :],
                            rhs=xcbf_sb[k][:, cs],
                            start=(k == 0), stop=(k == NCT - 1))
                    xdbl_part = bpool.tile([96, 512], bf16, tag=f"xdblp{h}")
                    nc.scalar.activation(xdbl_part[:], ps96[:, cs], AF.Copy)
                    # h1 chain on the Pool queue (idle until the scan) so the
                    # two allreduce chains don't serialize on SP
                    deng = nc.sync if h == 0 else nc.gpsimd
                    deng.dma_start(out=cc_in[h], in_=xdbl_part[:])
                    if profile_mode:
                        deng.dma_start(out=cc_out[h], in_=cc_in[h])
                    else:
                        nc.gpsimd.collective_compute(
                            "AllReduce", mybir.AluOpType.add, replica_groups=RG,
                            ins=[cc_in[h]], outs=[cc_out[h]])
                    deng.dma_start(out=xdbl_sb[:, cs], in_=cc_out[h])

                def z_half(h):
                    # write scaled z into sz; the Silu runs later in one
                    # batch (between softplus Lns and dA Exps) so its table
                    # loads never interleave with the Exp/Ln block
                    cs = slice(h * 512, (h + 1) * 512)
                    for m in range(4, 8):
                        ps = mchain(m, h)
                        j = m - 4
                        nc.vector.tensor_tensor(out=sz_sb[:, j, cs], in0=ps[:],
                                                in1=inv_bc[:, cs], op=OP.mult)

                rms_tail(0)
                for i in range(4, 8):
                    do_square(i)
                rms_tail(1)
                _mark("rmsT1")
                x_half(0)
                for i in range(4, 8):
                    do_transpose(i)
                tps.close()  # free psT banks for psD
                xproj_cc(0)
                _mark("inproj0")
                x_half(1)
                xproj_cc(1)
                _mark("inproj1")
                if debug:
                    nc.sync.dma_start(out=dbg["d_ss"].ap(), in_=ss_all[:])

                _mark("xproj")
                psD = ptp.enter_context(tc.tile_pool(name="psD", bufs=2, space="PSUM"))
                spool = ptp.enter_context(tc.tile_pool(name="spool", bufs=2))
                p1p = ptp.enter_context(tc.tile_pool(name="p1p", bufs=1))

                # dt_proj + softplus: softplus(u+b) = ln(1+exp(u+b)).
                p1v = [p1p.tile([128, T], f32, tag=f"p1v{j}", name=f"p1v{j}")
                       for j in range(NCT)]
                for h in range(2):
                    cs = slice(h * 512, (h + 1) * 512)
                    for j in range(NCT):
                        psd = psD.tile([128, 512], f32, tag="psD")
                        nc.tensor.matmul(
                            psd[:],
                            lhsT=wdt_sb[:, j * 128:(j + 1) * 128],
                            rhs=xdbl_sb[0:DT_RANK, cs],
                            start=True, stop=True)
                        e1 = spool.tile([128, 512], f32, tag="sp_e")
                        nc.scalar.activation(e1[:], psd[:], AF.Exp,
                                             bias=dtb_sb[:, j, :])
                        nc.vector.tensor_scalar_add(p1v[j][:, cs], e1[:], 1.0)
                for j in range(NCT):
                    for h in range(2):
                        cs = slice(h * 512, (h + 1) * 512)
                        nc.scalar.activation(dt_sb[:, j, cs], p1v[j][:, cs], AF.Ln)
                    nc.vector.tensor_tensor(
                        out=dtxc_sb[j][:], in0=dt_sb[:, j, :],
                        in1=xcbf_sb[j][:], op=OP.mult)
                # z m-chains after the dt block: lower PE priority, so the
                # psd matmuls preempt them when each allreduce half returns
                z_half(0)
                z_half(1)

            _mark("dt")
            if debug:
                nc.sync.dma_start(out=dbg["d_invbc"].ap(), in_=inv_bc[:])
                nc.sync.dma_start(out=dbg["d_hT"].ap(), in_=hTbf[:])
                nc.sync.dma_start(out=dbg["d_x0"].ap(), in_=x_sb[0][:])
                nc.sync.dma_start(out=dbg["d_xdbl"].ap(), in_=xdbl_sb[:])
        # ===================== scan block (n-pair outer, j-inner) =========
        with ExitStack() as back:
            woutp = back.enter_context(tc.tile_pool(name="woutp", bufs=1))
            wout_sb = woutp.tile([128, NCT, D_MODEL], f32r, tag="wout")

            p6 = back.enter_context(ExitStack())
            bcp = p6.enter_context(tc.tile_pool(name="bcp", bufs=3))
            dAp = p6.enter_context(tc.tile_pool(name="dAp", bufs=4))
            dBup = p6.enter_context(tc.tile_pool(name="dBup", bufs=4))
            hp = p6.enter_context(tc.tile_pool(name="hp", bufs=6))
            prp = p6.enter_context(tc.tile_pool(name="prp", bufs=4))
            gat = p6.enter_context(tc.tile_pool(name="gat", bufs=2))
            psY = p6.enter_context(tc.tile_pool(name="psY", bufs=1, space="PSUM"))

            psy = [psY.tile([128, T], f32, tag=f"psy{j}", name=f"psy{j}")
                   for j in range(NCT)]

            # States processed in pairs: [128, 2T] tiles with a zeroed seam
            # column resetting the scan at the segment boundary.
            NP = D_STATE // 2
            NPRE = 2
            bc_tiles = {}

            def load_bc(p):
                t = bcp.tile([128, 2, 2 * T], bf16, tag="bc")
                # free layout per B/C: [n-pair(2) x t-half(2) x 512];
                # broadcast rows of cc_out[2,96,512] to 128 partitions
                for bc_i in range(2):
                    base = t[:, bc_i, :]
                    for h in range(2):
                        nc.sync.dma_start(
                            out=bass.AP(tensor=base.tensor,
                                        offset=base.offset + h * 512,
                                        ap=[base.ap[0], [T, 2], [1, 512]]),
                            in_=bass.AP(
                                tensor=cc_out,
                                offset=96 * 512 * h + (64 + 16 * bc_i + 2 * p) * 512,
                                ap=[[0, 128], [512, 2], [1, 512]]))
                bc_tiles[p] = t

            def rep2(ap):
                return bass.AP(tensor=ap.tensor, offset=ap.offset,
                               ap=[ap.ap[0], [0, 2], ap.ap[1]])

            for p in range(NPRE):
                load_bc(p)

            # Scans are DVE-only (codegen rejects TensorScalarPtr on Pool).
            # DVE: dBu + scan (+ a few prods); Pool: most prods (plain TT).
            pending = []  # (j, p, h_tile)
            n_pend = [0]

            def flush_pending():
                for (j, p, hsc) in pending:
                    prod = prp.tile([128, 2 * T], bf16, tag="prod")
                    idx = n_pend[0]; n_pend[0] += 1
                    peng = nc.vector if idx % 6 == 5 else nc.gpsimd
                    peng.tensor_tensor(out=prod[:], in0=hsc[:],
                                       in1=bc_tiles[p][:, 1, :], op=OP.mult)
                    for s in range(2):
                        n = 2 * p + s
                        for h in range(2):
                            nc.tensor.matmul(
                                psy[j][:, h * 512:(h + 1) * 512],
                                lhsT=identbf_sb[:],
                                rhs=prod[:, s * T + h * 512:s * T + (h + 1) * 512],
                                start=(n == 0), stop=(n == D_STATE - 1))
                pending.clear()

            for p in range(NP):
                if p + NPRE < NP:
                    load_bc(p + NPRE)
                if p == 3:
                    # deferred z-gates: one Silu batch, in place, emitted
                    # once all softplus Exp/Ln work has long retired so the
                    # scheduler can't interleave it into that table set
                    nc.scalar.activation(sz_sb[:], sz_sb[:], AF.Silu)
                if p == 2:
                    # out_proj weights after the early B/C prefetches
                    nc.sync.dma_start(
                        out=wout_sb[:],
                        in_=w_out_t.ap().rearrange("(k p) n -> p k n", p=128))
                for j in range(NCT):
                    dA = dAp.tile([128, 2 * T], bf16, tag="dA")
                    for s in range(2):
                        nc.scalar.activation(dA[:, s * T:(s + 1) * T],
                                             dt_sb[:, j, :], AF.Exp,
                                             scale=a_sb[:, j, 2 * p + s:2 * p + s + 1])
                    nc.vector.memset(dA[:, T:T + 1], 0.0)  # segment seam
                    dBu = dBup.tile([128, 2 * T], bf16, tag="dBu")
                    nc.vector.tensor_tensor(out=dBu[:], in0=rep2(dtxc_sb[j][:]),
                                            in1=bc_tiles[p][:, 0, :], op=OP.mult)
                    hsc = hp.tile([128, 2 * T], bf16, tag="h")
                    nc.vector.tensor_tensor_scan(
                        hsc[:], dA[:], dBu[:], 0.0, OP.mult, OP.add)
                    pending.append((j, p, hsc))
                flush_pending()

            first = True
            for h in range(2):
                cs = slice(h * 512, (h + 1) * 512)
                for j in range(NCT):
                    t1 = gat.tile([128, 512], f32, tag="t1")
                    nc.vector.scalar_tensor_tensor(
                        out=t1[:], in0=xcbf_sb[j][:, cs], scalar=d_sb[:, j, :],
                        in1=psy[j][:, cs], op0=OP.mult, op1=OP.add)
                    # y2 on Pool (free at scan end) halves the PE-idle gap
                    nc.gpsimd.tensor_tensor(out=y2_sb[j][:, cs], in0=t1[:],
                                            in1=sz_sb[:, j, cs], op=OP.mult)
                    if first:
                        # keep the PE p-state hot through the y2 window by
                        # recycling the consumed psy[0] bank as scratch
                        first = False
                        for w in range(16):
                            nc.tensor.matmul(
                                psy[0][:, 0:512], lhsT=identbf_sb[:],
                                rhs=dtxc_sb[0][:, 0:512],
                                start=True, stop=True)

            _mark("scan")
            if debug:
                nc.sync.dma_start(out=dbg["d_sz0"].ap(), in_=sz_sb[:, 0, :])
                nc.sync.dma_start(out=dbg["d_dt"].ap(), in_=dt_sb[:])
                nc.sync.dma_start(out=dbg["d_y20"].ap(), in_=y2_sb[0][:])
            p6.close()  # frees scan pools + psY banks before out_proj
            # ================== out_proj partial ======================
            with ExitStack() as p7:
                psO = p7.enter_context(tc.tile_pool(name="psO", bufs=4, space="PSUM"))
                oev = p7.enter_context(tc.tile_pool(name="oev", bufs=2))
                for mp in range(4):  # m-pairs
                    ot = oev.tile([128, 2, T], bf16, tag="oev")
                    for mi in range(2):
                        m = mp * 2 + mi
                        for h in range(2):
                            pso = psO.tile([128, 512], f32, tag="psO")
                            for k in range(NCT):
                                nc.tensor.matmul(
                                    pso[:],
                                    lhsT=wout_sb[:, k, m * 128:(m + 1) * 128],
                                    rhs=y2_sb[k][:, h * 512:(h + 1) * 512],
                                    start=(k == 0), stop=(k == NCT - 1))
                            nc.scalar.activation(
                                ot[:, mi, h * 512:(h + 1) * 512], pso[:], AF.Copy)
                    nc.sync.dma_start(
                        out=bass.AP(tensor=part_out, offset=mp * 2 * 128 * T,
                                    ap=[[T, 128], [128 * T, 2], [1, T]]),
                        in_=ot[:])

    _mark("out_proj")
    nc.compile()
    return nc


def _get_program():
    if "nc" not in _CACHE:
        _CACHE["nc"] = _build_program()
    return _CACHE["nc"]


def kernel(hidden_states, norm_weight, in_proj_w, conv_w, conv_b, x_proj_w,
           dt_proj_w, dt_proj_b, A_log, D, out_proj_w):
    from concourse.bass_utils import run_bass_kernel_spmd
    import ml_dtypes

    bf = ml_dtypes.bfloat16

    hidden_states = np.asarray(hidden_states, dtype=np.float32)
    norm_weight = np.asarray(norm_weight, dtype=np.float32)
    in_proj_w = np.asarray(in_proj_w, dtype=np.float32)
    conv_w = np.asarray(conv_w, dtype=np.float32)
    conv_b = np.asarray(conv_b, dtype=np.float32)
    x_proj_w = np.asarray(x_proj_w, dtype=np.float32)
    dt_proj_w = np.asarray(dt_proj_w, dtype=np.float32)
    dt_proj_b = np.asarray(dt_proj_b, dtype=np.float32)
    A_log = np.asarray(A_log, dtype=np.float32)
    D = np.asarray(D, dtype=np.float32)
    out_proj_w = np.asarray(out_proj_w, dtype=np.float32)

    nc = _get_program()

    a_neg_full = -np.exp(A_log)  # [2048, 16]
    ident_bf = np.eye(128, dtype=bf)
    eye128 = np.eye(128, dtype=np.float32)

    in_maps = []
    for c in range(8):
        b, j = c // 4, c % 4
        sl = slice(CH * j, CH * (j + 1))
        w_in_cat = np.concatenate(
            [in_proj_w[sl], in_proj_w[D_INNER + CH * j:D_INNER + CH * (j + 1)]],
            axis=0)  # [2CH, D_MODEL]
        w_in_fold = w_in_cat * norm_weight[None, :]
        cpack = np.concatenate(
            [a_neg_full[sl], conv_w[sl], conv_b[sl, None], dt_proj_b[sl, None],
             D[sl, None]], axis=1).astype(np.float32)
        # diag_cw[p, j*512 + kk*128 + c] = conv_w[j*128+p, kk] * (c == p)
        dk = conv_w[sl].reshape(NCT, 128, D_CONV)            # [j, p, kk]
        diag = np.einsum('jpk,pc->pjkc', dk, eye128)         # [p, j, kk, c]
        in_maps.append({
            "hid_bf": hidden_states[b].astype(bf),
            "w_in_bf": np.ascontiguousarray(w_in_fold.T).astype(bf),
            "w_xp_bf": np.ascontiguousarray(x_proj_w[:, sl].T).astype(bf),
            "w_dt_bf": np.ascontiguousarray(dt_proj_w[sl, :].T).astype(bf),
            "w_out_T": np.ascontiguousarray(out_proj_w[:, sl].T),
            "cpack": np.ascontiguousarray(cpack),
            "ident_bf": ident_bf,
            "diag_cw": np.ascontiguousarray(diag.reshape(128, -1)).astype(bf),
        })

    import os
    kw = {}
    if os.environ.get("MAMBA_TRACE"):
        kw = dict(trace=True, tmpdir=os.environ.get("MAMBA_TRACE_DIR") or None)
    res = run_bass_kernel_spmd(nc, in_maps, list(range(8)), **kw)
    _CACHE["last_results"] = res

    out = np.zeros((2, T, D_MODEL), np.float32)
    for c in range(8):
        b = c // 4
        out[b] += res.results[c]["part_out"].T.astype(np.float32)
    return out, hidden_states
